# revision 84
# baseline (speedup 1.0000x reference)
"""3-layer GraphSAGE + classifier + log_softmax on 8 Trainium2 NeuronCores.

Self-contained: host-side sharding/packing + Bass/Tile device kernel.

Strategy
--------
concat([x, agg]) @ W  ==  x @ W_top + Ahat @ (x @ W_bot)   (linearity)
so aggregation happens in the 256-dim projected space.

- Nodes are permuted into 704 tiles of 128 (in-degree balanced), 88 tiles/core.
- Layer-1 phase A computes p = x@W1_bot first (two AllGathers fire as soon
  as each half of p is ready), then r = x@W1_top + b under the AG flight.
  Layers 2/3 have no separate phase A: their r/p are computed inside the
  previous layer's phase B, straight from the freshly-transposed x tiles in
  SBUF (x never round-trips DRAM), so each AllGather overlaps the previous
  layer's aggregation compute.
- Phase B (per group of 4 dst tiles): gather p[src] rows for in-edges via
  dma_gather (int16 indices; 4 windows, 2 per 45056-row AllGathered table).
  Windows 0/2 carry a fixed 384 edges per tile (zero padding); windows 1/3
  carry the remainder, chunk-padded with repeated indices (row-buffer-hit
  reads). Gathers map window -> SWDGE queue so all 4 Q7 core pairs generate
  descriptors concurrently; window-0/1 gathers are issued LA groups ahead
  to hide the AllGathers. The weighted one-hot selection matrices
  S[e, d] = wn_e * (dst_local_e == d) come half from the host (windows 0/2,
  dense DMA) and half built on DVE (windows 1/3, broadcast is_equal*mult);
  agg = sum_c S_c.T @ msg_c accumulates on the PE. x_next = relu(agg + r)
  (relu on the Scalar engine), transposed on the PE (2x 128x128).
- The classifier (768->7) accumulates incrementally: each layer's phase B
  adds x_l^T @ Wl[2l:2l+2] into the logits; log_softmax runs batched
  (max/exp/sum per 4-tile block, single ln at the end).
"""

import numpy as np

import concourse.bass as bass
import concourse.mybir as mybir
import concourse.tile as tile
from concourse import bacc
from concourse.bass_utils import run_bass_kernel_spmd
from concourse.masks import make_identity

# problem constants
N = 89250
IN_F = 500
HID = 256
NCLS = 7
FPAD = 512  # padded input feature dim

NC = 8  # cores
P = 128
NT = 704  # node tiles
TPC = NT // NC  # 88 tiles per core
NPAD = NT * P  # 90112
NPC = TPC * P  # 11264 nodes per core
HT = TPC // 2  # 44 tiles per half
QT = TPC // 4  # 22 tiles per quarter
QROWS = QT * P  # 2816 rows per quarter per core
TAB4 = NC * QROWS  # 22528 rows per AllGathered quarter-table
G4 = 4  # tiles per gather group
LA = 3  # lookahead groups for window-0/1 gathers (hide AllGather b)

f32 = mybir.dt.float32
f16 = mybir.dt.float16
i16 = mybir.dt.int16
i32 = mybir.dt.int32

_compile_cache = {}


# --------------------------------------------------------------------------
# host-side prep
# --------------------------------------------------------------------------

def _assign_tiles(in_deg):
    """LPT: assign node ids (0..NPAD) to (tile, slot), balancing in-edges."""
    import heapq

    order = np.argsort(-in_deg, kind="stable")
    heap = [(0, t) for t in range(NT)]
    heapq.heapify(heap)
    counts = np.zeros(NT, np.int32)
    newpos = np.empty(NPAD, np.int64)
    for v in order:
        load, t = heapq.heappop(heap)
        newpos[v] = t * P + counts[t]
        counts[t] += 1
        if counts[t] < P:
            heapq.heappush(heap, (load + int(in_deg[v]), t))
    return newpos


def _ru16(x):
    return (int(x) + 15) // 16 * 16


def prep(x, edge_index, edge_weight):
    src = edge_index[0].astype(np.int64)
    dst = edge_index[1].astype(np.int64)
    ew = edge_weight.astype(np.float32)

    cnt = np.bincount(dst, minlength=N).astype(np.float32)
    wn = ew / np.maximum(cnt[dst], 1.0)

    in_deg = np.zeros(NPAD, np.int64)
    in_deg[:N] = np.bincount(dst, minlength=N)
    newpos = _assign_tiles(in_deg)

    s2 = newpos[src]
    d2 = newpos[dst]
    # table row of each source: owner core c, local row jl; quarter q = which
    # 22-tile block of the core.  trow = c*QROWS + (jl mod QROWS)
    c_own = s2 // NPC
    jl = s2 % NPC
    q_of = jl // QROWS
    trow = c_own * QROWS + (jl % QROWS)

    tile_of = d2 // P
    dl = (d2 % P).astype(np.int64)

    order = np.argsort(tile_of, kind="stable")
    trow_o, q_o, dl_o, wn_o = trow[order], q_of[order], dl[order], wn[order]
    tile_o = tile_of[order]
    starts = np.searchsorted(tile_o, np.arange(NT + 1))

    # per (tile, window=quarter): sorted index lists + (dl, wn) in slot order
    tw_idx = [[None] * 4 for _ in range(NT)]
    tw_dl = [[None] * 4 for _ in range(NT)]
    tw_wn = [[None] * 4 for _ in range(NT)]
    for t in range(NT):
        lo, hi = starts[t], starts[t + 1]
        tr, qq = trow_o[lo:hi], q_o[lo:hi]
        dd, ww = dl_o[lo:hi], wn_o[lo:hi]
        for w in range(4):
            sel = np.nonzero(qq == w)[0]
            o = sel[np.argsort(tr[sel], kind="stable")]
            assert len(o) > 0, f"tile {t} quarter {w}: no edges"
            tw_idx[t][w] = tr[o].astype(np.int16)
            tw_dl[t][w] = dd[o]
            tw_wn[t][w] = ww[o]

    # per tile-slot chunk budgets (uniform across cores for SPMD); tiles are
    # chunk-aligned inside group gathers, so pad each (tile, window) to a
    # multiple of 128 with repeats of the last index (row-buffer-hit reads)
    cb = np.zeros((TPC, 4), np.int64)
    for tl in range(TPC):
        for w in range(4):
            mx = max(len(tw_idx[c * TPC + tl][w]) for c in range(NC))
            cb[tl, w] = (mx + P - 1) // P
    kc = cb.sum(axis=1)  # chunks per tile
    soff = np.zeros(TPC + 1, np.int64)
    soff[1:] = np.cumsum(kc)
    sct = int(soff[-1])

    # group-of-G4 gather packing: per (group, window) one gather whose index
    # list is the concat of the group's tiles (each padded to cb*128)
    ng4 = TPC // G4
    cb4 = np.zeros((ng4, 4), np.int64)  # chunks per (group, window)
    for g in range(ng4):
        cb4[g] = cb[g * G4:(g + 1) * G4].sum(axis=0)
    go4 = np.zeros((ng4, 5), np.int64)  # gidx column offsets (int16 cols)
    for g in range(ng4):
        go4[g, 1:] = np.cumsum(cb4[g] * 8)
    gmax = int(go4[:, 4].max())

    gidx = np.zeros((NC, ng4, P, gmax), np.int16)
    # compact S description (dst slot + weight per chunk/partition); dense
    # one-hot S chunks are built on-device by DVE broadcast compare
    mdl = np.zeros((NC, P, sct), np.float16)
    mwn = np.zeros((NC, P, sct), np.float16)
    for t in range(NT):
        c, tl = t // TPC, t % TPC
        g, ti = tl // G4, tl % G4
        ci0 = 0
        for w in range(4):
            idx = tw_idx[t][w]
            n = len(idx)
            b = int(cb[tl, w]) * P
            arr = np.full(b, idx[-1], np.int16)
            arr[:n] = idx
            wrapped = arr.reshape(-1, 16).T  # [16, b/16]
            coff = int(go4[g, w]) + int(cb[g * G4:tl, w].sum()) * 8
            gidx[c, g, :, coff:coff + b // 16] = np.tile(wrapped, (8, 1))
            sl = np.arange(n)
            mdl[c, sl % P, soff[tl] + ci0 + sl // P] = tw_dl[t][w]
            mwn[c, sl % P, soff[tl] + ci0 + sl // P] = tw_wn[t][w]
            ci0 += int(cb[tl, w])

    # transposed, padded, permuted node features
    xT = np.zeros((FPAD, NPAD), np.float16)
    xT[:IN_F, newpos[:N]] = x.T

    return {
        "newpos": newpos,
        "cb": cb,
        "cb4": cb4,
        "go4": go4,
        "kc": kc,
        "soff": soff,
        "sct": sct,
        "gmax": gmax,
        "xT": xT,
        "mdl": mdl,
        "mwn": mwn,
        "gidx": gidx,
    }


def pack_weights(W1, b1, W2, b2, W3, b3, Wl, bl):
    def chunk_rhs(W, kchunks, dtype):
        # [F, 512] -> [128, kchunks, 512]
        F = W.shape[0]
        Wp = np.zeros((kchunks * P, 512), np.float32)
        Wp[:F] = W
        return np.ascontiguousarray(
            Wp.reshape(kchunks, P, 512).transpose(1, 0, 2)
        ).astype(dtype)

    w1cat = np.concatenate([W1[:IN_F], W1[IN_F:]], axis=1)  # [500, 512]
    w2cat = np.concatenate([W2[:HID], W2[HID:]], axis=1)  # [256, 512]
    w3cat = np.concatenate([W3[:HID], W3[HID:]], axis=1)
    wl = np.ascontiguousarray(
        Wl.reshape(6, P, NCLS).transpose(1, 0, 2)
    ).astype(np.float16)  # [128, 6, 7]
    return {
        "w1": chunk_rhs(w1cat, 4, np.float16),
        "w2": chunk_rhs(w2cat, 2, np.float16),
        "w3": chunk_rhs(w3cat, 2, np.float16),
        "wl": wl,
        "b1": np.tile(b1[None, :], (P, 1)).astype(np.float32),
        "b2": np.tile(b2[None, :], (P, 1)).astype(np.float32),
        "b3": np.tile(b3[None, :], (P, 1)).astype(np.float32),
        "bl": np.tile(bl[None, :], (P, 1)).astype(np.float32),
    }


# --------------------------------------------------------------------------
# device kernel
# --------------------------------------------------------------------------

def build(struct):
    cb = struct["cb"]
    cb4 = struct["cb4"]
    go4 = struct["go4"]
    kc = struct["kc"]
    soff = struct["soff"]
    sct = struct["sct"]
    gmax = struct["gmax"]
    ng4 = TPC // G4
    cb4max = [int(cb4[:, w].max()) for w in range(4)]

    nc = bacc.Bacc(
        "TRN2", target_bir_lowering=False, debug=False, num_devices=NC,
        num_swdge_queues=4,
    )

    xTc = nc.dram_tensor("xTc", [FPAD, NPC], f16, kind="ExternalInput")
    mdl_d = nc.dram_tensor("mdl", [P, sct], f16, kind="ExternalInput")
    mwn_d = nc.dram_tensor("mwn", [P, sct], f16, kind="ExternalInput")
    gidx_d = nc.dram_tensor("gidx", [ng4, P, gmax], i16, kind="ExternalInput")
    w1_d = nc.dram_tensor("w1", [P, 4, 512], f16, kind="ExternalInput")
    w2_d = nc.dram_tensor("w2", [P, 2, 512], f16, kind="ExternalInput")
    w3_d = nc.dram_tensor("w3", [P, 2, 512], f16, kind="ExternalInput")
    wl_d = nc.dram_tensor("wl", [P, 6, NCLS], f16, kind="ExternalInput")
    b1_d = nc.dram_tensor("b1", [P, HID], f32, kind="ExternalInput")
    b2_d = nc.dram_tensor("b2", [P, HID], f32, kind="ExternalInput")
    b3_d = nc.dram_tensor("b3", [P, HID], f32, kind="ExternalInput")
    bl_d = nc.dram_tensor("bl", [P, NCLS], f32, kind="ExternalInput")
    out_d = nc.dram_tensor("out", [NPC, NCLS], f32, kind="ExternalOutput")

    with tile.TileContext(nc) as tc:
        with (
            tc.tile_pool(name="dram", bufs=1, space="DRAM") as dram,
            tc.tile_pool(name="const", bufs=1) as cpool,
            tc.tile_pool(name="lx", bufs=4) as lxpool,
            tc.tile_pool(name="stage", bufs=4) as stpool,
            tc.tile_pool(name="msga", bufs=LA + 1) as msgapool,
            tc.tile_pool(name="msgb", bufs=4) as msgbpool,
            tc.tile_pool(name="sbuild", bufs=3) as sbpool,
            tc.tile_pool(name="psa", bufs=2, space="PSUM") as psa,
            tc.tile_pool(name="psagg", bufs=3, space="PSUM") as psagg,
            tc.tile_pool(name="pstr", bufs=2, space="PSUM") as pstr,
                        tc.tile_pool(name="pscls", bufs=1, space="PSUM") as pscls,
        ):
            # ---- DRAM intermediates
            p_loc = [
                [dram.tile([QROWS, HID], f16, name=f"p{i}loc{h}")
                 for h in range(4)]
                for i in range(3)
            ]
            p_full = [
                [dram.tile([TAB4, HID], f16, addr_space="Shared",
                           name=f"p{i}full{h}") for h in range(4)]
                for i in range(3)
            ]
            r_dram = [dram.tile([NPC, HID], f16, name=f"r{i}d") for i in range(3)]

            # ---- constants
            w1_sb = cpool.tile([P, 4, 512], f16)
            nc.sync.dma_start(w1_sb[:], w1_d[:])
            w2_sb = cpool.tile([P, 2, 512], f16)
            nc.sync.dma_start(w2_sb[:], w2_d[:])
            w3_sb = cpool.tile([P, 2, 512], f16)
            nc.sync.dma_start(w3_sb[:], w3_d[:])
            wl_sb = cpool.tile([P, 6, NCLS], f16)
            nc.sync.dma_start(wl_sb[:], wl_d[:])
            b_sb = []
            for name, t in (("b1", b1_d), ("b2", b2_d), ("b3", b3_d)):
                bt = cpool.tile([P, HID], f32, name=name + "sb")
                nc.sync.dma_start(bt[:], t[:])
                b_sb.append(bt)
            bl_sb = cpool.tile([P, NCLS], f32)
            nc.sync.dma_start(bl_sb[:], bl_d[:])

            gidx_sb = cpool.tile([P, ng4, gmax], i16)
            nc.sync.dma_start(
                gidx_sb[:],
                gidx_d[:].rearrange("g p c -> p g c"),
            )

            ident = cpool.tile([P, P], f16)
            make_identity(nc, ident[:])

            mdl_sb = cpool.tile([P, sct], f16)
            nc.sync.dma_start(mdl_sb[:], mdl_d[:])
            mwn_sb = cpool.tile([P, sct], f16)
            nc.sync.dma_start(mwn_sb[:], mwn_d[:])
            iota_i = cpool.tile([P, P], i32)
            nc.gpsimd.iota(iota_i[:], pattern=[[1, P]], base=0,
                           channel_multiplier=0)
            iota_f = cpool.tile([P, P], f16)
            nc.vector.tensor_copy(iota_f[:], iota_i[:])

            lgall = cpool.tile([P, TPC, NCLS], f32)
            sm_all = cpool.tile([P, TPC], f32)

            # ---- layer-1 phase A (by pairs of tiles): r = x@Wtop+b, p = x@Wbot
            def pair_rows(buf, pr):
                return buf[pr * 2 * P:(pr + 1) * 2 * P, :].rearrange(
                    "(t p) c -> p t c", p=P
                )

            def phase_a_p(pr):
                """p = x @ W1_bot only — unblocks the AllGathers early."""
                half, lpr = pr // (QT // 2), pr % (QT // 2)
                lx = lxpool.tile([P, 4, 2 * P], f16, name="lx", tag="lx")
                nc.sync.dma_start(
                    lx[:],
                    xTc[:, pr * 2 * P:(pr + 1) * 2 * P].rearrange(
                        "(k p) c -> p k c", p=P
                    ),
                )
                pst = stpool.tile([P, 2, HID], f16, name="pst", tag="pst")
                for gt in range(2):
                    ps = psa.tile([P, HID], f32, name="psP", tag="psA")
                    for k in range(4):
                        nc.tensor.matmul(
                            out=ps[:], lhsT=lx[:, k, gt * P:(gt + 1) * P],
                            rhs=w1_sb[:, k, HID:],
                            start=(k == 0), stop=(k == 3),
                        )
                    nc.vector.tensor_copy(pst[:, gt, :], ps[:])
                nc.sync.dma_start(pair_rows(p_loc[0][half], lpr), pst[:])

            def phase_a_r(pr):
                """r = x @ W1_top + b — overlaps the AllGather flight."""
                lx = lxpool.tile([P, 4, 2 * P], f16, name="lx", tag="lx")
                nc.sync.dma_start(
                    lx[:],
                    xTc[:, pr * 2 * P:(pr + 1) * 2 * P].rearrange(
                        "(k p) c -> p k c", p=P
                    ),
                )
                rst = stpool.tile([P, 2, HID], f16, name="rst", tag="rst")
                for gt in range(2):
                    ps = psa.tile([P, HID], f32, name="psP", tag="psA")
                    for k in range(4):
                        nc.tensor.matmul(
                            out=ps[:], lhsT=lx[:, k, gt * P:(gt + 1) * P],
                            rhs=w1_sb[:, k, :HID],
                            start=(k == 0), stop=(k == 3),
                        )
                    nc.vector.tensor_tensor(
                        out=rst[:, gt, :], in0=ps[:], in1=b_sb[0][:],
                        op=mybir.AluOpType.add,
                    )
                nc.sync.dma_start(pair_rows(r_dram[0], pr), rst[:])

            def fire_ag(layer, half):
                nc.gpsimd.collective_compute(
                    "AllGather",
                    mybir.AluOpType.bypass,
                    replica_groups=[list(range(NC))],
                    ins=[p_loc[layer][half].opt()],
                    outs=[p_full[layer][half].opt()],
                )

            # ---- phase B by groups of G4 tiles
            def issue_gathers(layer, g, ws):
                ms = {}
                for w in ws:
                    pool = msgapool if w < 2 else msgbpool
                    m = pool.tile(
                        [P, cb4max[w], HID], f16, name=f"m{w}", tag=f"m{w}"
                    )
                    cols = int(cb4[g, w])
                    ni = cols * P
                    nc.gpsimd.dma_gather(
                        out_ap=m[:, :cols, :],
                        in_ap=p_full[layer][w][:, :],
                        idxs_ap=gidx_sb[:, g, int(go4[g, w]):int(go4[g, w + 1])],
                        num_idxs=ni,
                        num_idxs_reg=ni,
                        elem_size=HID,
                        single_packet=(ni <= 1024),
                        queue_num=w,
                    )
                    ms[w] = m
                return ms

            def phase_b_group(layer, g, msgs):
                last = layer == 2
                t0 = g * G4
                kcg = int(soff[t0 + G4] - soff[t0])
                o0 = int(soff[t0])
                s_v = sbpool.tile([P, kcg, P], f16, name="sv", tag="sv")
                nc.vector.tensor_tensor(
                    out=s_v[:],
                    in0=mdl_sb[:, o0:o0 + kcg].unsqueeze(2).broadcast_to(
                        [P, kcg, P]
                    ),
                    in1=iota_f[:].unsqueeze(1).broadcast_to([P, kcg, P]),
                    op=mybir.AluOpType.is_equal,
                )
                nc.vector.tensor_tensor(
                    out=s_v[:],
                    in0=s_v[:],
                    in1=mwn_sb[:, o0:o0 + kcg].unsqueeze(2).broadcast_to(
                        [P, kcg, P]
                    ),
                    op=mybir.AluOpType.mult,
                )
                rst = stpool.tile([P, G4, HID], f16, name="rl", tag="rl")
                nc.sync.dma_start(
                    rst[:],
                    r_dram[layer][t0 * P:(t0 + G4) * P, :].rearrange(
                        "(t p) c -> p t c", p=P
                    ),
                )
                aggs = []
                for ti in range(G4):
                    tl = t0 + ti
                    agg = psagg.tile([P, HID], f32, name="agg", tag="agg")
                    nchunks = int(kc[tl])
                    cv = int(soff[tl]) - o0
                    done = 0
                    for w in range(4):
                        cbo = int(cb[t0:tl, w].sum())
                        for j in range(int(cb[tl, w])):
                            nc.tensor.matmul(
                                out=agg[:],
                                lhsT=s_v[:, cv, :],
                                rhs=msgs[w][:, cbo + j, :],
                                start=(done == 0),
                                stop=(done == nchunks - 1),
                            )
                            cv += 1
                            done += 1
                    aggs.append(agg)
                for ti in range(G4):
                    tl = t0 + ti
                    agg = aggs[ti]
                    xsum = stpool.tile([P, HID], f16, name="xsum", tag="xsum")
                    nc.vector.tensor_tensor(
                        out=xsum[:], in0=agg[:], in1=rst[:, ti, :],
                        op=mybir.AluOpType.add,
                    )
                    xn = stpool.tile([P, HID], f16, name="xn", tag="xn")
                    nc.scalar.activation(
                        xn[:], xsum[:], mybir.ActivationFunctionType.Relu
                    )
                    xts = []
                    for h in range(2):
                        tp = pstr.tile([P, P], f16, name="tp", tag="tp")
                        nc.tensor.transpose(
                            out=tp[:], in_=xn[:, h * P:(h + 1) * P],
                            identity=ident[:],
                        )
                        xt = stpool.tile([P, P], f16, name=f"xt{h}",
                                         tag=f"xt{h}")
                        nc.vector.tensor_copy(xt[:], tp[:])
                        xts.append(xt)
                    # fused next-layer phase A: r/p for layer+1 from x in SBUF
                    if not last:
                        half, lt = tl // QT, tl % QT
                        wsb = w2_sb if layer == 0 else w3_sb
                        ps = psa.tile([P, 512], f32, name="psA", tag="psA")
                        for k in range(2):
                            nc.tensor.matmul(
                                out=ps[:], lhsT=xts[k][:], rhs=wsb[:, k, :],
                                start=(k == 0), stop=(k == 1),
                            )
                        rs2 = stpool.tile([P, HID], f16, name="rs2", tag="rs2")
                        nc.vector.tensor_tensor(
                            out=rs2[:], in0=ps[:, :HID], in1=b_sb[layer + 1][:],
                            op=mybir.AluOpType.add,
                        )
                        ps2 = stpool.tile([P, HID], f16, name="ps2", tag="ps2")
                        nc.vector.tensor_copy(ps2[:], ps[:, HID:])
                        nc.sync.dma_start(
                            r_dram[layer + 1][tl * P:(tl + 1) * P, :].rearrange(
                                "(o p) c -> p (o c)", p=P
                            ),
                            rs2[:],
                        )
                        nc.sync.dma_start(
                            p_loc[layer + 1][half][
                                lt * P:(lt + 1) * P, :
                            ].rearrange("(o p) c -> p (o c)", p=P),
                            ps2[:],
                        )
                    # incremental classifier: lgall += x_l^T @ Wl[2l:2l+2]
                    cls = pscls.tile([P, NCLS], f32, name="cls", tag="cls")
                    for kk in range(2):
                        nc.tensor.matmul(
                            out=cls[:], lhsT=xts[kk][:],
                            rhs=wl_sb[:, 2 * layer + kk, :],
                            start=(kk == 0), stop=(kk == 1),
                        )
                    if layer == 0:
                        nc.vector.tensor_copy(lgall[:, tl, :], cls[:])
                    else:
                        nc.vector.tensor_tensor(
                            out=lgall[:, tl, :], in0=lgall[:, tl, :],
                            in1=cls[:], op=mybir.AluOpType.add,
                        )
                    if last:
                        nc.vector.tensor_tensor(
                            out=lgall[:, tl, :], in0=lgall[:, tl, :],
                            in1=bl_sb[:], op=mybir.AluOpType.add,
                        )

            # batched shifted-exp over a block of tiles (layer 3)
            def softmax_block(t0, nt):
                mx = stpool.tile([P, nt], f32, name="mx", tag="mx")
                nc.vector.tensor_reduce(
                    out=mx[:], in_=lgall[:, t0:t0 + nt, :],
                    axis=mybir.AxisListType.X, op=mybir.AluOpType.max,
                )
                nc.vector.tensor_tensor(
                    out=lgall[:, t0:t0 + nt, :], in0=lgall[:, t0:t0 + nt, :],
                    in1=mx[:].unsqueeze(2).broadcast_to([P, nt, NCLS]),
                    op=mybir.AluOpType.subtract,
                )
                ex = stpool.tile([P, nt, NCLS], f32, name="ex", tag="ex")
                nc.scalar.activation(
                    ex[:], lgall[:, t0:t0 + nt, :],
                    mybir.ActivationFunctionType.Exp,
                )
                nc.vector.tensor_reduce(
                    out=sm_all[:, t0:t0 + nt], in_=ex[:],
                    axis=mybir.AxisListType.X, op=mybir.AluOpType.add,
                )

            # ---- drive: layer-1 phase A (p first, AGs early, r under AG),
            # then 3 fused phase-B passes
            for pr in range(HT):
                phase_a_p(pr)
                if pr % (QT // 2) == QT // 2 - 1:
                    fire_ag(0, pr // (QT // 2))
            for pr in range(HT):
                phase_a_r(pr)
            for layer in range(3):
                # lookahead gathers: windows 0/1 run LA groups ahead (their
                # quarter AGs finish first); windows 2/3 run one group ahead
                # so their desc-gen + DMA latency stays off the critical path
                amsgs = [issue_gathers(layer, g, (0, 1)) for g in range(LA)]
                bmsgs = [issue_gathers(layer, 0, (2, 3))]
                for g in range(ng4):
                    if g + 1 < ng4:
                        bmsgs.append(issue_gathers(layer, g + 1, (2, 3)))
                    if g + LA < ng4:
                        amsgs.append(issue_gathers(layer, g + LA, (0, 1)))
                    msgs = dict(bmsgs[g])
                    msgs.update(amsgs[g])
                    phase_b_group(layer, g, msgs)
                    if layer < 2:
                        if g in (5, 10, 16, 21):
                            fire_ag(layer + 1, {5: 0, 10: 1, 16: 2, 21: 3}[g])
                    else:
                        softmax_block(g * G4, G4)

            lsm = cpool.tile([P, TPC], f32)
            nc.scalar.activation(
                lsm[:], sm_all[:], mybir.ActivationFunctionType.Ln
            )
            nc.vector.tensor_tensor(
                out=lgall[:], in0=lgall[:],
                in1=lsm[:].unsqueeze(2).broadcast_to([P, TPC, NCLS]),
                op=mybir.AluOpType.subtract,
            )
            nc.sync.dma_start(
                out_d[:].rearrange("(t p) j -> p t j", p=P), lgall[:]
            )

    nc.compile()
    return nc


# --------------------------------------------------------------------------
# entry point
# --------------------------------------------------------------------------

def kernel(x, edge_index, edge_weight, W1, b1, W2, b2, W3, b3, Wl, bl):
    x = np.asarray(x, dtype=np.float32)
    edge_index = np.asarray(edge_index)
    edge_weight = np.asarray(edge_weight, dtype=np.float32)

    pp = prep(x, edge_index, edge_weight)
    wts = pack_weights(
        np.asarray(W1, np.float32), np.asarray(b1, np.float32),
        np.asarray(W2, np.float32), np.asarray(b2, np.float32),
        np.asarray(W3, np.float32), np.asarray(b3, np.float32),
        np.asarray(Wl, np.float32), np.asarray(bl, np.float32),
    )

    key = (pp["sct"], pp["gmax"], tuple(pp["cb"].reshape(-1).tolist()))
    if key not in _compile_cache:
        _compile_cache[key] = build(pp)
    nc = _compile_cache[key]

    in_maps = []
    for c in range(NC):
        in_maps.append({
            "xTc": np.ascontiguousarray(pp["xT"][:, c * NPC:(c + 1) * NPC]),
            "mdl": np.ascontiguousarray(pp["mdl"][c]),
            "mwn": np.ascontiguousarray(pp["mwn"][c]),
            "gidx": np.ascontiguousarray(pp["gidx"][c]),
            **wts,
        })

    res = run_bass_kernel_spmd(nc, in_maps, list(range(NC)))
    out_full = np.concatenate([res.results[c]["out"] for c in range(NC)], axis=0)
    return out_full[pp["newpos"][:N]].astype(np.float32)


if __name__ == "__main__":
    import time

    rng = np.random.default_rng(0)
    E = 899756
    ei = rng.integers(0, N, (2, E)).astype(np.int32)
    ew = rng.random(E, dtype=np.float32)
    x = rng.standard_normal((N, IN_F), dtype=np.float32)
    t0 = time.time()
    pp = prep(x, ei, ew)
    print("prep", time.time() - t0, "sct =", pp["sct"], "gmax =", pp["gmax"])


# revision 86
# speedup vs baseline: 1.0225x; 1.0225x over previous
"""3-layer GraphSAGE + classifier + log_softmax on 8 Trainium2 NeuronCores.

Self-contained: host-side sharding/packing + Bass/Tile device kernel.

Strategy
--------
concat([x, agg]) @ W  ==  x @ W_top + Ahat @ (x @ W_bot)   (linearity)
so aggregation happens in the 256-dim projected space.

- Nodes are permuted into 704 tiles of 128 (in-degree balanced), 88 tiles/core.
- Layer-1 phase A computes p = x@W1_bot first (two AllGathers fire as soon
  as each half of p is ready), then r = x@W1_top + b under the AG flight.
  Layers 2/3 have no separate phase A: their r/p are computed inside the
  previous layer's phase B, straight from the freshly-transposed x tiles in
  SBUF (x never round-trips DRAM), so each AllGather overlaps the previous
  layer's aggregation compute.
- Phase B (per group of 4 dst tiles): gather p[src] rows for in-edges via
  dma_gather (int16 indices; 4 windows, 2 per 45056-row AllGathered table).
  Windows 0/2 carry a fixed 384 edges per tile (zero padding); windows 1/3
  carry the remainder, chunk-padded with repeated indices (row-buffer-hit
  reads). Gathers map window -> SWDGE queue so all 4 Q7 core pairs generate
  descriptors concurrently; window-0/1 gathers are issued LA groups ahead
  to hide the AllGathers. The weighted one-hot selection matrices
  S[e, d] = wn_e * (dst_local_e == d) come half from the host (windows 0/2,
  dense DMA) and half built on DVE (windows 1/3, broadcast is_equal*mult);
  agg = sum_c S_c.T @ msg_c accumulates on the PE. x_next = relu(agg + r)
  (relu on the Scalar engine), transposed on the PE (2x 128x128).
- The classifier (768->7) accumulates incrementally: each layer's phase B
  adds x_l^T @ Wl[2l:2l+2] into the logits; log_softmax runs batched
  (max/exp/sum per 4-tile block, single ln at the end).
"""

import numpy as np

import concourse.bass as bass
import concourse.mybir as mybir
import concourse.tile as tile
from concourse import bacc
from concourse.bass_utils import run_bass_kernel_spmd
from concourse.masks import make_identity

# problem constants
N = 89250
IN_F = 500
HID = 256
NCLS = 7
FPAD = 512  # padded input feature dim

NC = 8  # cores
P = 128
NT = 704  # node tiles
TPC = NT // NC  # 88 tiles per core
NPAD = NT * P  # 90112
NPC = TPC * P  # 11264 nodes per core
HT = TPC // 2  # 44 tiles per half
QT = TPC // 4  # 22 tiles per quarter
QROWS = QT * P  # 2816 rows per quarter per core
TAB4 = NC * QROWS  # 22528 rows per AllGathered quarter-table
G4 = 4  # tiles per gather group
LA = 3  # lookahead groups for window-0/1 gathers (hide AllGather b)

f32 = mybir.dt.float32
f16 = mybir.dt.float16
i16 = mybir.dt.int16
i32 = mybir.dt.int32

_compile_cache = {}


# --------------------------------------------------------------------------
# host-side prep
# --------------------------------------------------------------------------

def _assign_tiles(in_deg):
    """LPT: assign node ids (0..NPAD) to (tile, slot), balancing in-edges."""
    import heapq

    order = np.argsort(-in_deg, kind="stable")
    heap = [(0, t) for t in range(NT)]
    heapq.heapify(heap)
    counts = np.zeros(NT, np.int32)
    newpos = np.empty(NPAD, np.int64)
    for v in order:
        load, t = heapq.heappop(heap)
        newpos[v] = t * P + counts[t]
        counts[t] += 1
        if counts[t] < P:
            heapq.heappush(heap, (load + int(in_deg[v]), t))
    return newpos


def _ru16(x):
    return (int(x) + 15) // 16 * 16


def prep(x, edge_index, edge_weight):
    src = edge_index[0].astype(np.int64)
    dst = edge_index[1].astype(np.int64)
    ew = edge_weight.astype(np.float32)

    cnt = np.bincount(dst, minlength=N).astype(np.float32)
    wn = ew / np.maximum(cnt[dst], 1.0)

    in_deg = np.zeros(NPAD, np.int64)
    in_deg[:N] = np.bincount(dst, minlength=N)
    newpos = _assign_tiles(in_deg)

    s2 = newpos[src]
    d2 = newpos[dst]
    # table row of each source: owner core c, local row jl; quarter q = which
    # 22-tile block of the core.  trow = c*QROWS + (jl mod QROWS)
    c_own = s2 // NPC
    jl = s2 % NPC
    q_of = jl // QROWS
    trow = c_own * QROWS + (jl % QROWS)

    tile_of = d2 // P
    dl = (d2 % P).astype(np.int64)

    order = np.argsort(tile_of, kind="stable")
    trow_o, q_o, dl_o, wn_o = trow[order], q_of[order], dl[order], wn[order]
    tile_o = tile_of[order]
    starts = np.searchsorted(tile_o, np.arange(NT + 1))

    # per (tile, window=quarter): sorted index lists + (dl, wn) in slot order
    tw_idx = [[None] * 4 for _ in range(NT)]
    tw_dl = [[None] * 4 for _ in range(NT)]
    tw_wn = [[None] * 4 for _ in range(NT)]
    for t in range(NT):
        lo, hi = starts[t], starts[t + 1]
        tr, qq = trow_o[lo:hi], q_o[lo:hi]
        dd, ww = dl_o[lo:hi], wn_o[lo:hi]
        for w in range(4):
            sel = np.nonzero(qq == w)[0]
            o = sel[np.argsort(tr[sel], kind="stable")]
            assert len(o) > 0, f"tile {t} quarter {w}: no edges"
            tw_idx[t][w] = tr[o].astype(np.int16)
            tw_dl[t][w] = dd[o]
            tw_wn[t][w] = ww[o]

    # per tile-slot chunk budgets (uniform across cores for SPMD); tiles are
    # chunk-aligned inside group gathers, so pad each (tile, window) to a
    # multiple of 128 with repeats of the last index (row-buffer-hit reads)
    cb = np.zeros((TPC, 4), np.int64)
    for tl in range(TPC):
        for w in range(4):
            mx = max(len(tw_idx[c * TPC + tl][w]) for c in range(NC))
            cb[tl, w] = (mx + P - 1) // P
    kc = cb.sum(axis=1)  # chunks per tile
    soff = np.zeros(TPC + 1, np.int64)
    soff[1:] = np.cumsum(kc)
    sct = int(soff[-1])

    # group-of-G4 gather packing: per (group, window) one gather whose index
    # list is the concat of the group's tiles (each padded to cb*128)
    ng4 = TPC // G4
    cb4 = np.zeros((ng4, 4), np.int64)  # chunks per (group, window)
    for g in range(ng4):
        cb4[g] = cb[g * G4:(g + 1) * G4].sum(axis=0)
    go4 = np.zeros((ng4, 5), np.int64)  # gidx column offsets (int16 cols)
    for g in range(ng4):
        go4[g, 1:] = np.cumsum(cb4[g] * 8)
    gmax = int(go4[:, 4].max())

    gidx = np.zeros((NC, ng4, P, gmax), np.int16)
    # compact S description (dst slot + weight per chunk/partition); dense
    # one-hot S chunks are built on-device by DVE broadcast compare
    mdl = np.zeros((NC, P, sct), np.float16)
    mwn = np.zeros((NC, P, sct), np.float16)
    for t in range(NT):
        c, tl = t // TPC, t % TPC
        g, ti = tl // G4, tl % G4
        ci0 = 0
        for w in range(4):
            idx = tw_idx[t][w]
            n = len(idx)
            b = int(cb[tl, w]) * P
            arr = np.full(b, idx[-1], np.int16)
            arr[:n] = idx
            wrapped = arr.reshape(-1, 16).T  # [16, b/16]
            coff = int(go4[g, w]) + int(cb[g * G4:tl, w].sum()) * 8
            gidx[c, g, :, coff:coff + b // 16] = np.tile(wrapped, (8, 1))
            sl = np.arange(n)
            mdl[c, sl % P, soff[tl] + ci0 + sl // P] = tw_dl[t][w]
            mwn[c, sl % P, soff[tl] + ci0 + sl // P] = tw_wn[t][w]
            ci0 += int(cb[tl, w])

    # transposed, padded, permuted node features
    xT = np.zeros((FPAD, NPAD), np.float16)
    xT[:IN_F, newpos[:N]] = x.T

    return {
        "newpos": newpos,
        "cb": cb,
        "cb4": cb4,
        "go4": go4,
        "kc": kc,
        "soff": soff,
        "sct": sct,
        "gmax": gmax,
        "xT": xT,
        "mdl": mdl,
        "mwn": mwn,
        "gidx": gidx,
    }


def pack_weights(W1, b1, W2, b2, W3, b3, Wl, bl):
    def chunk_rhs(W, kchunks, dtype):
        # [F, 512] -> [128, kchunks, 512]
        F = W.shape[0]
        Wp = np.zeros((kchunks * P, 512), np.float32)
        Wp[:F] = W
        return np.ascontiguousarray(
            Wp.reshape(kchunks, P, 512).transpose(1, 0, 2)
        ).astype(dtype)

    w1cat = np.concatenate([W1[:IN_F], W1[IN_F:]], axis=1)  # [500, 512]
    w2cat = np.concatenate([W2[:HID], W2[HID:]], axis=1)  # [256, 512]
    w3cat = np.concatenate([W3[:HID], W3[HID:]], axis=1)
    wl = np.ascontiguousarray(
        Wl.reshape(6, P, NCLS).transpose(1, 0, 2)
    ).astype(np.float16)  # [128, 6, 7]
    return {
        "w1": chunk_rhs(w1cat, 4, np.float16),
        "w2": chunk_rhs(w2cat, 2, np.float16),
        "w3": chunk_rhs(w3cat, 2, np.float16),
        "wl": wl,
        "b1": np.tile(b1[None, :], (P, 1)).astype(np.float32),
        "b2": np.tile(b2[None, :], (P, 1)).astype(np.float32),
        "b3": np.tile(b3[None, :], (P, 1)).astype(np.float32),
        "bl": np.tile(bl[None, :], (P, 1)).astype(np.float32),
    }


# --------------------------------------------------------------------------
# device kernel
# --------------------------------------------------------------------------

def build(struct):
    cb = struct["cb"]
    cb4 = struct["cb4"]
    go4 = struct["go4"]
    kc = struct["kc"]
    soff = struct["soff"]
    sct = struct["sct"]
    gmax = struct["gmax"]
    ng4 = TPC // G4
    cb4max = [int(cb4[:, w].max()) for w in range(4)]

    nc = bacc.Bacc(
        "TRN2", target_bir_lowering=False, debug=False, num_devices=NC,
        num_swdge_queues=4,
    )

    xTc = nc.dram_tensor("xTc", [FPAD, NPC], f16, kind="ExternalInput")
    mdl_d = nc.dram_tensor("mdl", [P, sct], f16, kind="ExternalInput")
    mwn_d = nc.dram_tensor("mwn", [P, sct], f16, kind="ExternalInput")
    gidx_d = nc.dram_tensor("gidx", [ng4, P, gmax], i16, kind="ExternalInput")
    w1_d = nc.dram_tensor("w1", [P, 4, 512], f16, kind="ExternalInput")
    w2_d = nc.dram_tensor("w2", [P, 2, 512], f16, kind="ExternalInput")
    w3_d = nc.dram_tensor("w3", [P, 2, 512], f16, kind="ExternalInput")
    wl_d = nc.dram_tensor("wl", [P, 6, NCLS], f16, kind="ExternalInput")
    b1_d = nc.dram_tensor("b1", [P, HID], f32, kind="ExternalInput")
    b2_d = nc.dram_tensor("b2", [P, HID], f32, kind="ExternalInput")
    b3_d = nc.dram_tensor("b3", [P, HID], f32, kind="ExternalInput")
    bl_d = nc.dram_tensor("bl", [P, NCLS], f32, kind="ExternalInput")
    out_d = nc.dram_tensor("out", [NPC, NCLS], f32, kind="ExternalOutput")

    with tile.TileContext(nc) as tc:
        with (
            tc.tile_pool(name="dram", bufs=1, space="DRAM") as dram,
            tc.tile_pool(name="const", bufs=1) as cpool,
            tc.tile_pool(name="lx", bufs=4) as lxpool,
            tc.tile_pool(name="stage", bufs=4) as stpool,
            tc.tile_pool(name="msga", bufs=LA + 1) as msgapool,
            tc.tile_pool(name="msgb", bufs=4) as msgbpool,
            tc.tile_pool(name="sbuild", bufs=2) as sbpool,
            tc.tile_pool(name="psa", bufs=2, space="PSUM") as psa,
            tc.tile_pool(name="psagg", bufs=2, space="PSUM") as psagg,
            tc.tile_pool(name="pstr", bufs=2, space="PSUM") as pstr,
                        tc.tile_pool(name="pscls", bufs=2, space="PSUM") as pscls,
        ):
            # ---- DRAM intermediates
            p_loc = [
                [dram.tile([QROWS, HID], f16, name=f"p{i}loc{h}")
                 for h in range(4)]
                for i in range(3)
            ]
            p_full = [
                [dram.tile([TAB4, HID], f16, addr_space="Shared",
                           name=f"p{i}full{h}") for h in range(4)]
                for i in range(3)
            ]
            r_dram = [dram.tile([NPC, HID], f16, name=f"r{i}d") for i in range(3)]

            # ---- constants
            w1_sb = cpool.tile([P, 4, 512], f16)
            nc.sync.dma_start(w1_sb[:], w1_d[:])
            w2_sb = cpool.tile([P, 2, 512], f16)
            nc.sync.dma_start(w2_sb[:], w2_d[:])
            w3_sb = cpool.tile([P, 2, 512], f16)
            nc.sync.dma_start(w3_sb[:], w3_d[:])
            wl_sb = cpool.tile([P, 6, NCLS], f16)
            nc.sync.dma_start(wl_sb[:], wl_d[:])
            b_sb = []
            for name, t in (("b1", b1_d), ("b2", b2_d), ("b3", b3_d)):
                bt = cpool.tile([P, HID], f32, name=name + "sb")
                nc.sync.dma_start(bt[:], t[:])
                b_sb.append(bt)
            bl_sb = cpool.tile([P, NCLS], f32)
            nc.sync.dma_start(bl_sb[:], bl_d[:])

            gidx_sb = cpool.tile([P, ng4, gmax], i16)
            nc.sync.dma_start(
                gidx_sb[:],
                gidx_d[:].rearrange("g p c -> p g c"),
            )

            ident = cpool.tile([P, P], f16)
            make_identity(nc, ident[:])

            mdl_sb = cpool.tile([P, sct], f16)
            nc.sync.dma_start(mdl_sb[:], mdl_d[:])
            mwn_sb = cpool.tile([P, sct], f16)
            nc.sync.dma_start(mwn_sb[:], mwn_d[:])
            iota_i = cpool.tile([P, P], i32)
            nc.gpsimd.iota(iota_i[:], pattern=[[1, P]], base=0,
                           channel_multiplier=0)
            iota_f = cpool.tile([P, P], f16)
            nc.vector.tensor_copy(iota_f[:], iota_i[:])

            lgall = cpool.tile([P, TPC, NCLS], f32)
            sm_all = cpool.tile([P, TPC], f32)

            # ---- layer-1 phase A (by pairs of tiles): r = x@Wtop+b, p = x@Wbot
            def pair_rows(buf, pr):
                return buf[pr * 2 * P:(pr + 1) * 2 * P, :].rearrange(
                    "(t p) c -> p t c", p=P
                )

            def phase_a_p(pr):
                """p = x @ W1_bot only — unblocks the AllGathers early."""
                half, lpr = pr // (QT // 2), pr % (QT // 2)
                lx = lxpool.tile([P, 4, 2 * P], f16, name="lx", tag="lx")
                nc.sync.dma_start(
                    lx[:],
                    xTc[:, pr * 2 * P:(pr + 1) * 2 * P].rearrange(
                        "(k p) c -> p k c", p=P
                    ),
                )
                pst = stpool.tile([P, 2, HID], f16, name="pst", tag="pst")
                for gt in range(2):
                    ps = psa.tile([P, HID], f32, name="psP", tag="psA")
                    for k in range(4):
                        nc.tensor.matmul(
                            out=ps[:], lhsT=lx[:, k, gt * P:(gt + 1) * P],
                            rhs=w1_sb[:, k, HID:],
                            start=(k == 0), stop=(k == 3),
                        )
                    nc.vector.tensor_copy(pst[:, gt, :], ps[:])
                nc.sync.dma_start(pair_rows(p_loc[0][half], lpr), pst[:])

            def phase_a_r(pr):
                """r = x @ W1_top + b — overlaps the AllGather flight."""
                lx = lxpool.tile([P, 4, 2 * P], f16, name="lx", tag="lx")
                nc.sync.dma_start(
                    lx[:],
                    xTc[:, pr * 2 * P:(pr + 1) * 2 * P].rearrange(
                        "(k p) c -> p k c", p=P
                    ),
                )
                rst = stpool.tile([P, 2, HID], f16, name="rst", tag="rst")
                for gt in range(2):
                    ps = psa.tile([P, HID], f32, name="psP", tag="psA")
                    for k in range(4):
                        nc.tensor.matmul(
                            out=ps[:], lhsT=lx[:, k, gt * P:(gt + 1) * P],
                            rhs=w1_sb[:, k, :HID],
                            start=(k == 0), stop=(k == 3),
                        )
                    nc.vector.tensor_tensor(
                        out=rst[:, gt, :], in0=ps[:], in1=b_sb[0][:],
                        op=mybir.AluOpType.add,
                    )
                nc.sync.dma_start(pair_rows(r_dram[0], pr), rst[:])

            def fire_ag(layer, half):
                nc.gpsimd.collective_compute(
                    "AllGather",
                    mybir.AluOpType.bypass,
                    replica_groups=[list(range(NC))],
                    ins=[p_loc[layer][half].opt()],
                    outs=[p_full[layer][half].opt()],
                )

            # ---- phase B by groups of G4 tiles
            def issue_gathers(layer, g, ws):
                ms = {}
                for w in ws:
                    pool = msgapool if w < 2 else msgbpool
                    m = pool.tile(
                        [P, cb4max[w], HID], f16, name=f"m{w}", tag=f"m{w}"
                    )
                    cols = int(cb4[g, w])
                    ni = cols * P
                    nc.gpsimd.dma_gather(
                        out_ap=m[:, :cols, :],
                        in_ap=p_full[layer][w][:, :],
                        idxs_ap=gidx_sb[:, g, int(go4[g, w]):int(go4[g, w + 1])],
                        num_idxs=ni,
                        num_idxs_reg=ni,
                        elem_size=HID,
                        single_packet=(ni <= 1024),
                        queue_num=w,
                    )
                    ms[w] = m
                return ms

            def phase_b_group(layer, g, msgs):
                last = layer == 2
                t0 = g * G4
                kcg = int(soff[t0 + G4] - soff[t0])
                o0 = int(soff[t0])
                s_v = sbpool.tile([P, kcg, P], f16, name="sv", tag="sv")
                nc.vector.tensor_tensor(
                    out=s_v[:],
                    in0=mdl_sb[:, o0:o0 + kcg].unsqueeze(2).broadcast_to(
                        [P, kcg, P]
                    ),
                    in1=iota_f[:].unsqueeze(1).broadcast_to([P, kcg, P]),
                    op=mybir.AluOpType.is_equal,
                )
                nc.vector.tensor_tensor(
                    out=s_v[:],
                    in0=s_v[:],
                    in1=mwn_sb[:, o0:o0 + kcg].unsqueeze(2).broadcast_to(
                        [P, kcg, P]
                    ),
                    op=mybir.AluOpType.mult,
                )
                rst = stpool.tile([P, G4, HID], f16, name="rl", tag="rl")
                nc.sync.dma_start(
                    rst[:],
                    r_dram[layer][t0 * P:(t0 + G4) * P, :].rearrange(
                        "(t p) c -> p t c", p=P
                    ),
                )
                aggs = []
                for ti in range(G4):
                    tl = t0 + ti
                    agg = psagg.tile([P, HID], f32, name="agg", tag="agg")
                    nchunks = int(kc[tl])
                    cv = int(soff[tl]) - o0
                    done = 0
                    for w in range(4):
                        cbo = int(cb[t0:tl, w].sum())
                        for j in range(int(cb[tl, w])):
                            nc.tensor.matmul(
                                out=agg[:],
                                lhsT=s_v[:, cv, :],
                                rhs=msgs[w][:, cbo + j, :],
                                start=(done == 0),
                                stop=(done == nchunks - 1),
                            )
                            cv += 1
                            done += 1
                    aggs.append(agg)
                for ti in range(G4):
                    tl = t0 + ti
                    agg = aggs[ti]
                    xsum = stpool.tile([P, HID], f16, name="xsum", tag="xsum")
                    nc.vector.tensor_tensor(
                        out=xsum[:], in0=agg[:], in1=rst[:, ti, :],
                        op=mybir.AluOpType.add,
                    )
                    xn = stpool.tile([P, HID], f16, name="xn", tag="xn")
                    nc.scalar.activation(
                        xn[:], xsum[:], mybir.ActivationFunctionType.Relu
                    )
                    xts = []
                    for h in range(2):
                        tp = pstr.tile([P, P], f16, name="tp", tag="tp")
                        nc.tensor.transpose(
                            out=tp[:], in_=xn[:, h * P:(h + 1) * P],
                            identity=ident[:],
                        )
                        xt = stpool.tile([P, P], f16, name=f"xt{h}",
                                         tag=f"xt{h}")
                        nc.vector.tensor_copy(xt[:], tp[:])
                        xts.append(xt)
                    # fused next-layer phase A: r/p for layer+1 from x in SBUF
                    if not last:
                        half, lt = tl // QT, tl % QT
                        wsb = w2_sb if layer == 0 else w3_sb
                        ps = psa.tile([P, 512], f32, name="psA", tag="psA")
                        for k in range(2):
                            nc.tensor.matmul(
                                out=ps[:], lhsT=xts[k][:], rhs=wsb[:, k, :],
                                start=(k == 0), stop=(k == 1),
                            )
                        rs2 = stpool.tile([P, HID], f16, name="rs2", tag="rs2")
                        nc.vector.tensor_tensor(
                            out=rs2[:], in0=ps[:, :HID], in1=b_sb[layer + 1][:],
                            op=mybir.AluOpType.add,
                        )
                        ps2 = stpool.tile([P, HID], f16, name="ps2", tag="ps2")
                        nc.vector.tensor_copy(ps2[:], ps[:, HID:])
                        nc.sync.dma_start(
                            r_dram[layer + 1][tl * P:(tl + 1) * P, :].rearrange(
                                "(o p) c -> p (o c)", p=P
                            ),
                            rs2[:],
                        )
                        nc.sync.dma_start(
                            p_loc[layer + 1][half][
                                lt * P:(lt + 1) * P, :
                            ].rearrange("(o p) c -> p (o c)", p=P),
                            ps2[:],
                        )
                    # incremental classifier: lgall += x_l^T @ Wl[2l:2l+2]
                    cls = pscls.tile([P, NCLS], f32, name="cls", tag="cls")
                    for kk in range(2):
                        nc.tensor.matmul(
                            out=cls[:], lhsT=xts[kk][:],
                            rhs=wl_sb[:, 2 * layer + kk, :],
                            start=(kk == 0), stop=(kk == 1),
                        )
                    if layer == 0:
                        nc.vector.tensor_copy(lgall[:, tl, :], cls[:])
                    else:
                        nc.vector.tensor_tensor(
                            out=lgall[:, tl, :], in0=lgall[:, tl, :],
                            in1=cls[:], op=mybir.AluOpType.add,
                        )
                    if last:
                        nc.vector.tensor_tensor(
                            out=lgall[:, tl, :], in0=lgall[:, tl, :],
                            in1=bl_sb[:], op=mybir.AluOpType.add,
                        )

            # batched shifted-exp over a block of tiles (layer 3)
            def softmax_block(t0, nt):
                mx = stpool.tile([P, nt], f32, name="mx", tag="mx")
                nc.vector.tensor_reduce(
                    out=mx[:], in_=lgall[:, t0:t0 + nt, :],
                    axis=mybir.AxisListType.X, op=mybir.AluOpType.max,
                )
                nc.vector.tensor_tensor(
                    out=lgall[:, t0:t0 + nt, :], in0=lgall[:, t0:t0 + nt, :],
                    in1=mx[:].unsqueeze(2).broadcast_to([P, nt, NCLS]),
                    op=mybir.AluOpType.subtract,
                )
                ex = stpool.tile([P, nt, NCLS], f32, name="ex", tag="ex")
                nc.scalar.activation(
                    ex[:], lgall[:, t0:t0 + nt, :],
                    mybir.ActivationFunctionType.Exp,
                )
                nc.vector.tensor_reduce(
                    out=sm_all[:, t0:t0 + nt], in_=ex[:],
                    axis=mybir.AxisListType.X, op=mybir.AluOpType.add,
                )

            # finalize log-softmax + output DMA for a block of tiles
            def emit_out(t0, nt):
                lsm = cpool.tile([P, nt], f32, name=f"lsm{t0}")
                nc.scalar.activation(
                    lsm[:], sm_all[:, t0:t0 + nt],
                    mybir.ActivationFunctionType.Ln,
                )
                nc.vector.tensor_tensor(
                    out=lgall[:, t0:t0 + nt, :], in0=lgall[:, t0:t0 + nt, :],
                    in1=lsm[:].unsqueeze(2).broadcast_to([P, nt, NCLS]),
                    op=mybir.AluOpType.subtract,
                )
                nc.sync.dma_start(
                    out_d[t0 * P:(t0 + nt) * P, :].rearrange(
                        "(t p) j -> p t j", p=P
                    ),
                    lgall[:, t0:t0 + nt, :],
                )

            # ---- drive: layer-1 phase A (p first, AGs early, r under AG),
            # then 3 fused phase-B passes
            for pr in range(HT):
                phase_a_p(pr)
                if pr % (QT // 2) == QT // 2 - 1:
                    fire_ag(0, pr // (QT // 2))
            for pr in range(HT):
                phase_a_r(pr)
            for layer in range(3):
                # lookahead gathers: windows 0/1 run LA groups ahead (their
                # quarter AGs finish first); windows 2/3 run one group ahead
                # so their desc-gen + DMA latency stays off the critical path
                amsgs = [issue_gathers(layer, g, (0, 1)) for g in range(LA)]
                bmsgs = [issue_gathers(layer, 0, (2, 3))]
                for g in range(ng4):
                    if g + 1 < ng4:
                        bmsgs.append(issue_gathers(layer, g + 1, (2, 3)))
                    if g + LA < ng4:
                        amsgs.append(issue_gathers(layer, g + LA, (0, 1)))
                    msgs = dict(bmsgs[g])
                    msgs.update(amsgs[g])
                    phase_b_group(layer, g, msgs)
                    if layer < 2:
                        if g in (5, 10, 16, 21):
                            fire_ag(layer + 1, {5: 0, 10: 1, 16: 2, 21: 3}[g])
                    else:
                        softmax_block(g * G4, G4)
                        if g == HT // G4 - 1:
                            emit_out(0, HT)

            emit_out(HT, HT)

    nc.compile()
    return nc


# --------------------------------------------------------------------------
# entry point
# --------------------------------------------------------------------------

def kernel(x, edge_index, edge_weight, W1, b1, W2, b2, W3, b3, Wl, bl):
    x = np.asarray(x, dtype=np.float32)
    edge_index = np.asarray(edge_index)
    edge_weight = np.asarray(edge_weight, dtype=np.float32)

    pp = prep(x, edge_index, edge_weight)
    wts = pack_weights(
        np.asarray(W1, np.float32), np.asarray(b1, np.float32),
        np.asarray(W2, np.float32), np.asarray(b2, np.float32),
        np.asarray(W3, np.float32), np.asarray(b3, np.float32),
        np.asarray(Wl, np.float32), np.asarray(bl, np.float32),
    )

    key = (pp["sct"], pp["gmax"], tuple(pp["cb"].reshape(-1).tolist()))
    if key not in _compile_cache:
        _compile_cache[key] = build(pp)
    nc = _compile_cache[key]

    in_maps = []
    for c in range(NC):
        in_maps.append({
            "xTc": np.ascontiguousarray(pp["xT"][:, c * NPC:(c + 1) * NPC]),
            "mdl": np.ascontiguousarray(pp["mdl"][c]),
            "mwn": np.ascontiguousarray(pp["mwn"][c]),
            "gidx": np.ascontiguousarray(pp["gidx"][c]),
            **wts,
        })

    res = run_bass_kernel_spmd(nc, in_maps, list(range(NC)))
    out_full = np.concatenate([res.results[c]["out"] for c in range(NC)], axis=0)
    return out_full[pp["newpos"][:N]].astype(np.float32)


if __name__ == "__main__":
    import time

    rng = np.random.default_rng(0)
    E = 899756
    ei = rng.integers(0, N, (2, E)).astype(np.int32)
    ew = rng.random(E, dtype=np.float32)
    x = rng.standard_normal((N, IN_F), dtype=np.float32)
    t0 = time.time()
    pp = prep(x, ei, ew)
    print("prep", time.time() - t0, "sct =", pp["sct"], "gmax =", pp["gmax"])


# revision 88
# speedup vs baseline: 1.0249x; 1.0023x over previous
"""3-layer GraphSAGE + classifier + log_softmax on 8 Trainium2 NeuronCores.

Self-contained: host-side sharding/packing + Bass/Tile device kernel.

Strategy
--------
concat([x, agg]) @ W  ==  x @ W_top + Ahat @ (x @ W_bot)   (linearity)
so aggregation happens in the 256-dim projected space.

- Nodes are permuted into 704 tiles of 128 (in-degree balanced), 88 tiles/core.
- Layer-1 phase A computes p = x@W1_bot first (two AllGathers fire as soon
  as each half of p is ready), then r = x@W1_top + b under the AG flight.
  Layers 2/3 have no separate phase A: their r/p are computed inside the
  previous layer's phase B, straight from the freshly-transposed x tiles in
  SBUF (x never round-trips DRAM), so each AllGather overlaps the previous
  layer's aggregation compute.
- Phase B (per group of 4 dst tiles): gather p[src] rows for in-edges via
  dma_gather (int16 indices; 4 windows, 2 per 45056-row AllGathered table).
  Windows 0/2 carry a fixed 384 edges per tile (zero padding); windows 1/3
  carry the remainder, chunk-padded with repeated indices (row-buffer-hit
  reads). Gathers map window -> SWDGE queue so all 4 Q7 core pairs generate
  descriptors concurrently; window-0/1 gathers are issued LA groups ahead
  to hide the AllGathers. The weighted one-hot selection matrices
  S[e, d] = wn_e * (dst_local_e == d) come half from the host (windows 0/2,
  dense DMA) and half built on DVE (windows 1/3, broadcast is_equal*mult);
  agg = sum_c S_c.T @ msg_c accumulates on the PE. x_next = relu(agg + r)
  (relu on the Scalar engine), transposed on the PE (2x 128x128).
- The classifier (768->7) accumulates incrementally: each layer's phase B
  adds x_l^T @ Wl[2l:2l+2] into the logits; log_softmax runs batched
  (max/exp/sum per 4-tile block, single ln at the end).
"""

import numpy as np

import concourse.bass as bass
import concourse.mybir as mybir
import concourse.tile as tile
from concourse import bacc
from concourse.bass_utils import run_bass_kernel_spmd
from concourse.masks import make_identity

# problem constants
N = 89250
IN_F = 500
HID = 256
NCLS = 7
FPAD = 512  # padded input feature dim

NC = 8  # cores
P = 128
NT = 704  # node tiles
TPC = NT // NC  # 88 tiles per core
NPAD = NT * P  # 90112
NPC = TPC * P  # 11264 nodes per core
HT = TPC // 2  # 44 tiles per half
QT = TPC // 4  # 22 tiles per quarter
QROWS = QT * P  # 2816 rows per quarter per core
TAB4 = NC * QROWS  # 22528 rows per AllGathered quarter-table
G4 = 4  # tiles per gather group
LA = 3  # lookahead groups for window-0/1 gathers (hide AllGather b)

f32 = mybir.dt.float32
f16 = mybir.dt.float16
i16 = mybir.dt.int16
i32 = mybir.dt.int32

_compile_cache = {}


# --------------------------------------------------------------------------
# host-side prep
# --------------------------------------------------------------------------

def _assign_tiles(in_deg):
    """LPT: assign node ids (0..NPAD) to (tile, slot), balancing in-edges."""
    import heapq

    order = np.argsort(-in_deg, kind="stable")
    heap = [(0, t) for t in range(NT)]
    heapq.heapify(heap)
    counts = np.zeros(NT, np.int32)
    newpos = np.empty(NPAD, np.int64)
    for v in order:
        load, t = heapq.heappop(heap)
        newpos[v] = t * P + counts[t]
        counts[t] += 1
        if counts[t] < P:
            heapq.heappush(heap, (load + int(in_deg[v]), t))
    return newpos


def _ru16(x):
    return (int(x) + 15) // 16 * 16


def prep(x, edge_index, edge_weight):
    src = edge_index[0].astype(np.int64)
    dst = edge_index[1].astype(np.int64)
    ew = edge_weight.astype(np.float32)

    cnt = np.bincount(dst, minlength=N).astype(np.float32)
    wn = ew / np.maximum(cnt[dst], 1.0)

    in_deg = np.zeros(NPAD, np.int64)
    in_deg[:N] = np.bincount(dst, minlength=N)
    newpos = _assign_tiles(in_deg)

    s2 = newpos[src]
    d2 = newpos[dst]
    # table row of each source: owner core c, local row jl; quarter q = which
    # 22-tile block of the core.  trow = c*QROWS + (jl mod QROWS)
    c_own = s2 // NPC
    jl = s2 % NPC
    q_of = jl // QROWS
    trow = c_own * QROWS + (jl % QROWS)

    tile_of = d2 // P
    dl = (d2 % P).astype(np.int64)

    order = np.argsort(tile_of, kind="stable")
    trow_o, q_o, dl_o, wn_o = trow[order], q_of[order], dl[order], wn[order]
    tile_o = tile_of[order]
    starts = np.searchsorted(tile_o, np.arange(NT + 1))

    # per (tile, window=quarter): sorted index lists + (dl, wn) in slot order
    tw_idx = [[None] * 4 for _ in range(NT)]
    tw_dl = [[None] * 4 for _ in range(NT)]
    tw_wn = [[None] * 4 for _ in range(NT)]
    for t in range(NT):
        lo, hi = starts[t], starts[t + 1]
        tr, qq = trow_o[lo:hi], q_o[lo:hi]
        dd, ww = dl_o[lo:hi], wn_o[lo:hi]
        for w in range(4):
            sel = np.nonzero(qq == w)[0]
            o = sel[np.argsort(tr[sel], kind="stable")]
            assert len(o) > 0, f"tile {t} quarter {w}: no edges"
            tw_idx[t][w] = tr[o].astype(np.int16)
            tw_dl[t][w] = dd[o]
            tw_wn[t][w] = ww[o]

    # per tile-slot chunk budgets (uniform across cores for SPMD); tiles are
    # chunk-aligned inside group gathers, so pad each (tile, window) to a
    # multiple of 128 with repeats of the last index (row-buffer-hit reads)
    cb = np.zeros((TPC, 4), np.int64)
    for tl in range(TPC):
        for w in range(4):
            mx = max(len(tw_idx[c * TPC + tl][w]) for c in range(NC))
            cb[tl, w] = (mx + P - 1) // P
    kc = cb.sum(axis=1)  # chunks per tile
    soff = np.zeros(TPC + 1, np.int64)
    soff[1:] = np.cumsum(kc)
    sct = int(soff[-1])

    # group-of-G4 gather packing: per (group, window) one gather whose index
    # list is the concat of the group's tiles (each padded to cb*128)
    ng4 = TPC // G4
    cb4 = np.zeros((ng4, 4), np.int64)  # chunks per (group, window)
    for g in range(ng4):
        cb4[g] = cb[g * G4:(g + 1) * G4].sum(axis=0)
    go4 = np.zeros((ng4, 5), np.int64)  # gidx column offsets (int16 cols)
    for g in range(ng4):
        go4[g, 1:] = np.cumsum(cb4[g] * 8)
    gmax = int(go4[:, 4].max())

    gidx = np.zeros((NC, ng4, P, gmax), np.int16)
    # compact S description (dst slot + weight per chunk/partition); dense
    # one-hot S chunks are built on-device by DVE broadcast compare
    mdl = np.zeros((NC, P, sct), np.float16)
    mwn = np.zeros((NC, P, sct), np.float16)
    for t in range(NT):
        c, tl = t // TPC, t % TPC
        g, ti = tl // G4, tl % G4
        ci0 = 0
        for w in range(4):
            idx = tw_idx[t][w]
            n = len(idx)
            b = int(cb[tl, w]) * P
            arr = np.full(b, idx[-1], np.int16)
            arr[:n] = idx
            wrapped = arr.reshape(-1, 16).T  # [16, b/16]
            coff = int(go4[g, w]) + int(cb[g * G4:tl, w].sum()) * 8
            gidx[c, g, :, coff:coff + b // 16] = np.tile(wrapped, (8, 1))
            sl = np.arange(n)
            mdl[c, sl % P, soff[tl] + ci0 + sl // P] = tw_dl[t][w]
            mwn[c, sl % P, soff[tl] + ci0 + sl // P] = tw_wn[t][w]
            ci0 += int(cb[tl, w])

    # transposed, padded, permuted node features
    xT = np.zeros((FPAD, NPAD), np.float16)
    xT[:IN_F, newpos[:N]] = x.T

    return {
        "newpos": newpos,
        "cb": cb,
        "cb4": cb4,
        "go4": go4,
        "kc": kc,
        "soff": soff,
        "sct": sct,
        "gmax": gmax,
        "xT": xT,
        "mdl": mdl,
        "mwn": mwn,
        "gidx": gidx,
    }


def pack_weights(W1, b1, W2, b2, W3, b3, Wl, bl):
    def chunk_rhs(W, kchunks, dtype):
        # [F, 512] -> [128, kchunks, 512]
        F = W.shape[0]
        Wp = np.zeros((kchunks * P, 512), np.float32)
        Wp[:F] = W
        return np.ascontiguousarray(
            Wp.reshape(kchunks, P, 512).transpose(1, 0, 2)
        ).astype(dtype)

    w1cat = np.concatenate([W1[:IN_F], W1[IN_F:]], axis=1)  # [500, 512]
    w2cat = np.concatenate([W2[:HID], W2[HID:]], axis=1)  # [256, 512]
    w3cat = np.concatenate([W3[:HID], W3[HID:]], axis=1)
    wl = np.ascontiguousarray(
        Wl.reshape(6, P, NCLS).transpose(1, 0, 2)
    ).astype(np.float16)  # [128, 6, 7]
    return {
        "w1": chunk_rhs(w1cat, 4, np.float16),
        "w2": chunk_rhs(w2cat, 2, np.float16),
        "w3": chunk_rhs(w3cat, 2, np.float16),
        "wl": wl,
        "b1": np.tile(b1[None, :], (P, 1)).astype(np.float32),
        "b2": np.tile(b2[None, :], (P, 1)).astype(np.float32),
        "b3": np.tile(b3[None, :], (P, 1)).astype(np.float32),
        "bl": np.tile(bl[None, :], (P, 1)).astype(np.float32),
    }


# --------------------------------------------------------------------------
# device kernel
# --------------------------------------------------------------------------

def build(struct):
    cb = struct["cb"]
    cb4 = struct["cb4"]
    go4 = struct["go4"]
    kc = struct["kc"]
    soff = struct["soff"]
    sct = struct["sct"]
    gmax = struct["gmax"]
    ng4 = TPC // G4
    cb4max = [int(cb4[:, w].max()) for w in range(4)]

    nc = bacc.Bacc(
        "TRN2", target_bir_lowering=False, debug=False, num_devices=NC,
        num_swdge_queues=4,
    )

    xTc = nc.dram_tensor("xTc", [FPAD, NPC], f16, kind="ExternalInput")
    mdl_d = nc.dram_tensor("mdl", [P, sct], f16, kind="ExternalInput")
    mwn_d = nc.dram_tensor("mwn", [P, sct], f16, kind="ExternalInput")
    gidx_d = nc.dram_tensor("gidx", [ng4, P, gmax], i16, kind="ExternalInput")
    w1_d = nc.dram_tensor("w1", [P, 4, 512], f16, kind="ExternalInput")
    w2_d = nc.dram_tensor("w2", [P, 2, 512], f16, kind="ExternalInput")
    w3_d = nc.dram_tensor("w3", [P, 2, 512], f16, kind="ExternalInput")
    wl_d = nc.dram_tensor("wl", [P, 6, NCLS], f16, kind="ExternalInput")
    b1_d = nc.dram_tensor("b1", [P, HID], f32, kind="ExternalInput")
    b2_d = nc.dram_tensor("b2", [P, HID], f32, kind="ExternalInput")
    b3_d = nc.dram_tensor("b3", [P, HID], f32, kind="ExternalInput")
    bl_d = nc.dram_tensor("bl", [P, NCLS], f32, kind="ExternalInput")
    out_d = nc.dram_tensor("out", [NPC, NCLS], f32, kind="ExternalOutput")

    with tile.TileContext(nc) as tc:
        with (
            tc.tile_pool(name="dram", bufs=1, space="DRAM") as dram,
            tc.tile_pool(name="const", bufs=1) as cpool,
            tc.tile_pool(name="lx", bufs=4) as lxpool,
            tc.tile_pool(name="stage", bufs=4) as stpool,
            tc.tile_pool(name="msga", bufs=LA + 1) as msgapool,
            tc.tile_pool(name="msgb", bufs=4) as msgbpool,
            tc.tile_pool(name="sbuild", bufs=2) as sbpool,
            tc.tile_pool(name="psa", bufs=2, space="PSUM") as psa,
            tc.tile_pool(name="psagg", bufs=2, space="PSUM") as psagg,
            tc.tile_pool(name="pstr", bufs=2, space="PSUM") as pstr,
                        tc.tile_pool(name="pscls", bufs=2, space="PSUM") as pscls,
        ):
            # ---- DRAM intermediates
            p_loc = [
                [dram.tile([QROWS, HID], f16, name=f"p{i}loc{h}")
                 for h in range(4)]
                for i in range(3)
            ]
            p_full = [
                [dram.tile([TAB4, HID], f16, addr_space="Shared",
                           name=f"p{i}full{h}") for h in range(4)]
                for i in range(3)
            ]
            r_dram = [dram.tile([NPC, HID], f16, name=f"r{i}d") for i in range(3)]

            # ---- constants
            w1_sb = cpool.tile([P, 4, 512], f16)
            nc.sync.dma_start(w1_sb[:], w1_d[:])
            w2_sb = cpool.tile([P, 2, 512], f16)
            nc.sync.dma_start(w2_sb[:], w2_d[:])
            w3_sb = cpool.tile([P, 2, 512], f16)
            nc.sync.dma_start(w3_sb[:], w3_d[:])
            wl_sb = cpool.tile([P, 6, NCLS], f16)
            nc.sync.dma_start(wl_sb[:], wl_d[:])
            b_sb = []
            for name, t in (("b1", b1_d), ("b2", b2_d), ("b3", b3_d)):
                bt = cpool.tile([P, HID], f32, name=name + "sb")
                nc.sync.dma_start(bt[:], t[:])
                b_sb.append(bt)
            bl_sb = cpool.tile([P, NCLS], f32)
            nc.sync.dma_start(bl_sb[:], bl_d[:])

            gidx_sb = cpool.tile([P, ng4, gmax], i16)
            nc.sync.dma_start(
                gidx_sb[:],
                gidx_d[:].rearrange("g p c -> p g c"),
            )

            ident = cpool.tile([P, P], f16)
            make_identity(nc, ident[:])

            mdl_sb = cpool.tile([P, sct], f16)
            nc.sync.dma_start(mdl_sb[:], mdl_d[:])
            mwn_sb = cpool.tile([P, sct], f16)
            nc.sync.dma_start(mwn_sb[:], mwn_d[:])
            iota_i = cpool.tile([P, P], i32)
            nc.gpsimd.iota(iota_i[:], pattern=[[1, P]], base=0,
                           channel_multiplier=0)
            iota_f = cpool.tile([P, P], f16)
            nc.vector.tensor_copy(iota_f[:], iota_i[:])

            lgall = cpool.tile([P, TPC, NCLS], f32)
            sm_all = cpool.tile([P, TPC], f32)

            # ---- layer-1 phase A (by pairs of tiles): r = x@Wtop+b, p = x@Wbot
            def pair_rows(buf, pr):
                return buf[pr * 2 * P:(pr + 1) * 2 * P, :].rearrange(
                    "(t p) c -> p t c", p=P
                )

            def phase_a_p(pr):
                """p = x @ W1_bot only — unblocks the AllGathers early."""
                half, lpr = pr // (QT // 2), pr % (QT // 2)
                lx = lxpool.tile([P, 4, 2 * P], f16, name="lx", tag="lx")
                nc.sync.dma_start(
                    lx[:],
                    xTc[:, pr * 2 * P:(pr + 1) * 2 * P].rearrange(
                        "(k p) c -> p k c", p=P
                    ),
                )
                pst = stpool.tile([P, 2, HID], f16, name="pst", tag="pst")
                for gt in range(2):
                    ps = psa.tile([P, HID], f32, name="psP", tag="psA")
                    for k in range(4):
                        nc.tensor.matmul(
                            out=ps[:], lhsT=lx[:, k, gt * P:(gt + 1) * P],
                            rhs=w1_sb[:, k, HID:],
                            start=(k == 0), stop=(k == 3),
                        )
                    nc.vector.tensor_copy(pst[:, gt, :], ps[:])
                nc.sync.dma_start(pair_rows(p_loc[0][half], lpr), pst[:])

            def phase_a_r(pr):
                """r = x @ W1_top + b — overlaps the AllGather flight."""
                lx = lxpool.tile([P, 4, 2 * P], f16, name="lx", tag="lx")
                nc.sync.dma_start(
                    lx[:],
                    xTc[:, pr * 2 * P:(pr + 1) * 2 * P].rearrange(
                        "(k p) c -> p k c", p=P
                    ),
                )
                rst = stpool.tile([P, 2, HID], f16, name="rst", tag="rst")
                for gt in range(2):
                    ps = psa.tile([P, HID], f32, name="psP", tag="psA")
                    for k in range(4):
                        nc.tensor.matmul(
                            out=ps[:], lhsT=lx[:, k, gt * P:(gt + 1) * P],
                            rhs=w1_sb[:, k, :HID],
                            start=(k == 0), stop=(k == 3),
                        )
                    nc.vector.tensor_tensor(
                        out=rst[:, gt, :], in0=ps[:], in1=b_sb[0][:],
                        op=mybir.AluOpType.add,
                    )
                nc.sync.dma_start(pair_rows(r_dram[0], pr), rst[:])

            def fire_ag(layer, half):
                nc.gpsimd.collective_compute(
                    "AllGather",
                    mybir.AluOpType.bypass,
                    replica_groups=[list(range(NC))],
                    ins=[p_loc[layer][half].opt()],
                    outs=[p_full[layer][half].opt()],
                )

            # ---- phase B by groups of G4 tiles
            def issue_gathers(layer, g, ws):
                ms = {}
                for w in ws:
                    pool = msgapool if w < 2 else msgbpool
                    m = pool.tile(
                        [P, cb4max[w], HID], f16, name=f"m{w}", tag=f"m{w}"
                    )
                    cols = int(cb4[g, w])
                    ni = cols * P
                    nc.gpsimd.dma_gather(
                        out_ap=m[:, :cols, :],
                        in_ap=p_full[layer][w][:, :],
                        idxs_ap=gidx_sb[:, g, int(go4[g, w]):int(go4[g, w + 1])],
                        num_idxs=ni,
                        num_idxs_reg=ni,
                        elem_size=HID,
                        single_packet=(ni <= 1024),
                        queue_num=(g + w) % 4,
                    )
                    ms[w] = m
                return ms

            def phase_b_group(layer, g, msgs):
                last = layer == 2
                t0 = g * G4
                kcg = int(soff[t0 + G4] - soff[t0])
                o0 = int(soff[t0])
                s_v = sbpool.tile([P, kcg, P], f16, name="sv", tag="sv")
                nc.vector.tensor_tensor(
                    out=s_v[:],
                    in0=mdl_sb[:, o0:o0 + kcg].unsqueeze(2).broadcast_to(
                        [P, kcg, P]
                    ),
                    in1=iota_f[:].unsqueeze(1).broadcast_to([P, kcg, P]),
                    op=mybir.AluOpType.is_equal,
                )
                nc.vector.tensor_tensor(
                    out=s_v[:],
                    in0=s_v[:],
                    in1=mwn_sb[:, o0:o0 + kcg].unsqueeze(2).broadcast_to(
                        [P, kcg, P]
                    ),
                    op=mybir.AluOpType.mult,
                )
                rst = stpool.tile([P, G4, HID], f16, name="rl", tag="rl")
                nc.sync.dma_start(
                    rst[:],
                    r_dram[layer][t0 * P:(t0 + G4) * P, :].rearrange(
                        "(t p) c -> p t c", p=P
                    ),
                )
                aggs = []
                for ti in range(G4):
                    tl = t0 + ti
                    agg = psagg.tile([P, HID], f32, name="agg", tag="agg")
                    nchunks = int(kc[tl])
                    cv = int(soff[tl]) - o0
                    done = 0
                    for w in range(4):
                        cbo = int(cb[t0:tl, w].sum())
                        for j in range(int(cb[tl, w])):
                            nc.tensor.matmul(
                                out=agg[:],
                                lhsT=s_v[:, cv, :],
                                rhs=msgs[w][:, cbo + j, :],
                                start=(done == 0),
                                stop=(done == nchunks - 1),
                            )
                            cv += 1
                            done += 1
                    aggs.append(agg)
                for ti in range(G4):
                    tl = t0 + ti
                    agg = aggs[ti]
                    xsum = stpool.tile([P, HID], f16, name="xsum", tag="xsum")
                    nc.vector.tensor_tensor(
                        out=xsum[:], in0=agg[:], in1=rst[:, ti, :],
                        op=mybir.AluOpType.add,
                    )
                    xn = stpool.tile([P, HID], f16, name="xn", tag="xn")
                    nc.scalar.activation(
                        xn[:], xsum[:], mybir.ActivationFunctionType.Relu
                    )
                    xts = []
                    for h in range(2):
                        tp = pstr.tile([P, P], f16, name="tp", tag="tp")
                        nc.tensor.transpose(
                            out=tp[:], in_=xn[:, h * P:(h + 1) * P],
                            identity=ident[:],
                        )
                        xt = stpool.tile([P, P], f16, name=f"xt{h}",
                                         tag=f"xt{h}")
                        nc.vector.tensor_copy(xt[:], tp[:])
                        xts.append(xt)
                    # fused next-layer phase A: r/p for layer+1 from x in SBUF
                    if not last:
                        half, lt = tl // QT, tl % QT
                        wsb = w2_sb if layer == 0 else w3_sb
                        ps = psa.tile([P, 512], f32, name="psA", tag="psA")
                        for k in range(2):
                            nc.tensor.matmul(
                                out=ps[:], lhsT=xts[k][:], rhs=wsb[:, k, :],
                                start=(k == 0), stop=(k == 1),
                            )
                        rs2 = stpool.tile([P, HID], f16, name="rs2", tag="rs2")
                        nc.vector.tensor_tensor(
                            out=rs2[:], in0=ps[:, :HID], in1=b_sb[layer + 1][:],
                            op=mybir.AluOpType.add,
                        )
                        ps2 = stpool.tile([P, HID], f16, name="ps2", tag="ps2")
                        nc.vector.tensor_copy(ps2[:], ps[:, HID:])
                        nc.sync.dma_start(
                            r_dram[layer + 1][tl * P:(tl + 1) * P, :].rearrange(
                                "(o p) c -> p (o c)", p=P
                            ),
                            rs2[:],
                        )
                        nc.sync.dma_start(
                            p_loc[layer + 1][half][
                                lt * P:(lt + 1) * P, :
                            ].rearrange("(o p) c -> p (o c)", p=P),
                            ps2[:],
                        )
                    # incremental classifier: lgall += x_l^T @ Wl[2l:2l+2]
                    cls = pscls.tile([P, NCLS], f32, name="cls", tag="cls")
                    for kk in range(2):
                        nc.tensor.matmul(
                            out=cls[:], lhsT=xts[kk][:],
                            rhs=wl_sb[:, 2 * layer + kk, :],
                            start=(kk == 0), stop=(kk == 1),
                        )
                    if layer == 0:
                        nc.vector.tensor_copy(lgall[:, tl, :], cls[:])
                    else:
                        nc.vector.tensor_tensor(
                            out=lgall[:, tl, :], in0=lgall[:, tl, :],
                            in1=cls[:], op=mybir.AluOpType.add,
                        )
                    if last:
                        nc.vector.tensor_tensor(
                            out=lgall[:, tl, :], in0=lgall[:, tl, :],
                            in1=bl_sb[:], op=mybir.AluOpType.add,
                        )

            # batched shifted-exp over a block of tiles (layer 3)
            def softmax_block(t0, nt):
                mx = stpool.tile([P, nt], f32, name="mx", tag="mx")
                nc.vector.tensor_reduce(
                    out=mx[:], in_=lgall[:, t0:t0 + nt, :],
                    axis=mybir.AxisListType.X, op=mybir.AluOpType.max,
                )
                nc.vector.tensor_tensor(
                    out=lgall[:, t0:t0 + nt, :], in0=lgall[:, t0:t0 + nt, :],
                    in1=mx[:].unsqueeze(2).broadcast_to([P, nt, NCLS]),
                    op=mybir.AluOpType.subtract,
                )
                ex = stpool.tile([P, nt, NCLS], f32, name="ex", tag="ex")
                nc.scalar.activation(
                    ex[:], lgall[:, t0:t0 + nt, :],
                    mybir.ActivationFunctionType.Exp,
                )
                nc.vector.tensor_reduce(
                    out=sm_all[:, t0:t0 + nt], in_=ex[:],
                    axis=mybir.AxisListType.X, op=mybir.AluOpType.add,
                )

            # ---- drive: layer-1 phase A (p first, AGs early, r under AG),
            # then 3 fused phase-B passes
            for pr in range(HT):
                phase_a_p(pr)
                if pr % (QT // 2) == QT // 2 - 1:
                    fire_ag(0, pr // (QT // 2))
            for pr in range(HT):
                phase_a_r(pr)
            for layer in range(3):
                # lookahead gathers: windows 0/1 run LA groups ahead (their
                # quarter AGs finish first); windows 2/3 run one group ahead
                # so their desc-gen + DMA latency stays off the critical path
                amsgs = [issue_gathers(layer, g, (0, 1)) for g in range(LA)]
                bmsgs = [issue_gathers(layer, 0, (2, 3))]
                for g in range(ng4):
                    if g + 1 < ng4:
                        bmsgs.append(issue_gathers(layer, g + 1, (2, 3)))
                    if g + LA < ng4:
                        amsgs.append(issue_gathers(layer, g + LA, (0, 1)))
                    msgs = dict(bmsgs[g])
                    msgs.update(amsgs[g])
                    phase_b_group(layer, g, msgs)
                    if layer < 2:
                        if g in (5, 10, 16, 21):
                            fire_ag(layer + 1, {5: 0, 10: 1, 16: 2, 21: 3}[g])
                    else:
                        softmax_block(g * G4, G4)

            lsm = cpool.tile([P, TPC], f32)
            nc.scalar.activation(
                lsm[:], sm_all[:], mybir.ActivationFunctionType.Ln
            )
            nc.vector.tensor_tensor(
                out=lgall[:], in0=lgall[:],
                in1=lsm[:].unsqueeze(2).broadcast_to([P, TPC, NCLS]),
                op=mybir.AluOpType.subtract,
            )
            nc.sync.dma_start(
                out_d[:].rearrange("(t p) j -> p t j", p=P), lgall[:]
            )

    nc.compile()
    return nc


# --------------------------------------------------------------------------
# entry point
# --------------------------------------------------------------------------

def kernel(x, edge_index, edge_weight, W1, b1, W2, b2, W3, b3, Wl, bl):
    x = np.asarray(x, dtype=np.float32)
    edge_index = np.asarray(edge_index)
    edge_weight = np.asarray(edge_weight, dtype=np.float32)

    pp = prep(x, edge_index, edge_weight)
    wts = pack_weights(
        np.asarray(W1, np.float32), np.asarray(b1, np.float32),
        np.asarray(W2, np.float32), np.asarray(b2, np.float32),
        np.asarray(W3, np.float32), np.asarray(b3, np.float32),
        np.asarray(Wl, np.float32), np.asarray(bl, np.float32),
    )

    key = (pp["sct"], pp["gmax"], tuple(pp["cb"].reshape(-1).tolist()))
    if key not in _compile_cache:
        _compile_cache[key] = build(pp)
    nc = _compile_cache[key]

    in_maps = []
    for c in range(NC):
        in_maps.append({
            "xTc": np.ascontiguousarray(pp["xT"][:, c * NPC:(c + 1) * NPC]),
            "mdl": np.ascontiguousarray(pp["mdl"][c]),
            "mwn": np.ascontiguousarray(pp["mwn"][c]),
            "gidx": np.ascontiguousarray(pp["gidx"][c]),
            **wts,
        })

    res = run_bass_kernel_spmd(nc, in_maps, list(range(NC)))
    out_full = np.concatenate([res.results[c]["out"] for c in range(NC)], axis=0)
    return out_full[pp["newpos"][:N]].astype(np.float32)


if __name__ == "__main__":
    import time

    rng = np.random.default_rng(0)
    E = 899756
    ei = rng.integers(0, N, (2, E)).astype(np.int32)
    ew = rng.random(E, dtype=np.float32)
    x = rng.standard_normal((N, IN_F), dtype=np.float32)
    t0 = time.time()
    pp = prep(x, ei, ew)
    print("prep", time.time() - t0, "sct =", pp["sct"], "gmax =", pp["gmax"])


# revision 89
# speedup vs baseline: 1.0261x; 1.0011x over previous
"""3-layer GraphSAGE + classifier + log_softmax on 8 Trainium2 NeuronCores.

Self-contained: host-side sharding/packing + Bass/Tile device kernel.

Strategy
--------
concat([x, agg]) @ W  ==  x @ W_top + Ahat @ (x @ W_bot)   (linearity)
so aggregation happens in the 256-dim projected space.

- Nodes are permuted into 704 tiles of 128 (in-degree balanced), 88 tiles/core.
- Layer-1 phase A computes p = x@W1_bot first (two AllGathers fire as soon
  as each half of p is ready), then r = x@W1_top + b under the AG flight.
  Layers 2/3 have no separate phase A: their r/p are computed inside the
  previous layer's phase B, straight from the freshly-transposed x tiles in
  SBUF (x never round-trips DRAM), so each AllGather overlaps the previous
  layer's aggregation compute.
- Phase B (per group of 4 dst tiles): gather p[src] rows for in-edges via
  dma_gather (int16 indices; 4 windows, 2 per 45056-row AllGathered table).
  Windows 0/2 carry a fixed 384 edges per tile (zero padding); windows 1/3
  carry the remainder, chunk-padded with repeated indices (row-buffer-hit
  reads). Gathers map window -> SWDGE queue so all 4 Q7 core pairs generate
  descriptors concurrently; window-0/1 gathers are issued LA groups ahead
  to hide the AllGathers. The weighted one-hot selection matrices
  S[e, d] = wn_e * (dst_local_e == d) come half from the host (windows 0/2,
  dense DMA) and half built on DVE (windows 1/3, broadcast is_equal*mult);
  agg = sum_c S_c.T @ msg_c accumulates on the PE. x_next = relu(agg + r)
  (relu on the Scalar engine), transposed on the PE (2x 128x128).
- The classifier (768->7) accumulates incrementally: each layer's phase B
  adds x_l^T @ Wl[2l:2l+2] into the logits; log_softmax runs batched
  (max/exp/sum per 4-tile block, single ln at the end).
"""

import numpy as np

import concourse.bass as bass
import concourse.mybir as mybir
import concourse.tile as tile
from concourse import bacc
from concourse.bass_utils import run_bass_kernel_spmd
from concourse.masks import make_identity

# problem constants
N = 89250
IN_F = 500
HID = 256
NCLS = 7
FPAD = 512  # padded input feature dim

NC = 8  # cores
P = 128
NT = 704  # node tiles
TPC = NT // NC  # 88 tiles per core
NPAD = NT * P  # 90112
NPC = TPC * P  # 11264 nodes per core
HT = TPC // 2  # 44 tiles per half
QT = TPC // 4  # 22 tiles per quarter
QROWS = QT * P  # 2816 rows per quarter per core
TAB4 = NC * QROWS  # 22528 rows per AllGathered quarter-table
G4 = 4  # tiles per gather group
LA = 3  # lookahead groups for window-0/1 gathers (hide AllGather b)

f32 = mybir.dt.float32
f16 = mybir.dt.float16
i16 = mybir.dt.int16
i32 = mybir.dt.int32

_compile_cache = {}


# --------------------------------------------------------------------------
# host-side prep
# --------------------------------------------------------------------------

def _assign_tiles(in_deg):
    """LPT: assign node ids (0..NPAD) to (tile, slot), balancing in-edges."""
    import heapq

    order = np.argsort(-in_deg, kind="stable")
    heap = [(0, t) for t in range(NT)]
    heapq.heapify(heap)
    counts = np.zeros(NT, np.int32)
    newpos = np.empty(NPAD, np.int64)
    for v in order:
        load, t = heapq.heappop(heap)
        newpos[v] = t * P + counts[t]
        counts[t] += 1
        if counts[t] < P:
            heapq.heappush(heap, (load + int(in_deg[v]), t))
    return newpos


def _ru16(x):
    return (int(x) + 15) // 16 * 16


def prep(x, edge_index, edge_weight):
    src = edge_index[0].astype(np.int64)
    dst = edge_index[1].astype(np.int64)
    ew = edge_weight.astype(np.float32)

    cnt = np.bincount(dst, minlength=N).astype(np.float32)
    wn = ew / np.maximum(cnt[dst], 1.0)

    in_deg = np.zeros(NPAD, np.int64)
    in_deg[:N] = np.bincount(dst, minlength=N)
    newpos = _assign_tiles(in_deg)

    s2 = newpos[src]
    d2 = newpos[dst]
    # table row of each source: owner core c, local row jl; quarter q = which
    # 22-tile block of the core.  trow = c*QROWS + (jl mod QROWS)
    c_own = s2 // NPC
    jl = s2 % NPC
    q_of = jl // QROWS
    trow = c_own * QROWS + (jl % QROWS)

    tile_of = d2 // P
    dl = (d2 % P).astype(np.int64)

    order = np.argsort(tile_of, kind="stable")
    trow_o, q_o, dl_o, wn_o = trow[order], q_of[order], dl[order], wn[order]
    tile_o = tile_of[order]
    starts = np.searchsorted(tile_o, np.arange(NT + 1))

    # per (tile, window=quarter): sorted index lists + (dl, wn) in slot order
    tw_idx = [[None] * 4 for _ in range(NT)]
    tw_dl = [[None] * 4 for _ in range(NT)]
    tw_wn = [[None] * 4 for _ in range(NT)]
    for t in range(NT):
        lo, hi = starts[t], starts[t + 1]
        tr, qq = trow_o[lo:hi], q_o[lo:hi]
        dd, ww = dl_o[lo:hi], wn_o[lo:hi]
        for w in range(4):
            sel = np.nonzero(qq == w)[0]
            o = sel[np.argsort(tr[sel], kind="stable")]
            assert len(o) > 0, f"tile {t} quarter {w}: no edges"
            tw_idx[t][w] = tr[o].astype(np.int16)
            tw_dl[t][w] = dd[o]
            tw_wn[t][w] = ww[o]

    # per tile-slot chunk budgets (uniform across cores for SPMD); tiles are
    # chunk-aligned inside group gathers, so pad each (tile, window) to a
    # multiple of 128 with repeats of the last index (row-buffer-hit reads)
    cb = np.zeros((TPC, 4), np.int64)
    for tl in range(TPC):
        for w in range(4):
            mx = max(len(tw_idx[c * TPC + tl][w]) for c in range(NC))
            cb[tl, w] = (mx + P - 1) // P
    kc = cb.sum(axis=1)  # chunks per tile
    soff = np.zeros(TPC + 1, np.int64)
    soff[1:] = np.cumsum(kc)
    sct = int(soff[-1])

    # group-of-G4 gather packing: per (group, window) one gather whose index
    # list is the concat of the group's tiles (each padded to cb*128)
    ng4 = TPC // G4
    cb4 = np.zeros((ng4, 4), np.int64)  # chunks per (group, window)
    for g in range(ng4):
        cb4[g] = cb[g * G4:(g + 1) * G4].sum(axis=0)
    go4 = np.zeros((ng4, 5), np.int64)  # gidx column offsets (int16 cols)
    for g in range(ng4):
        go4[g, 1:] = np.cumsum(cb4[g] * 8)
    gmax = int(go4[:, 4].max())

    gidx = np.zeros((NC, ng4, P, gmax), np.int16)
    # compact S description (dst slot + weight per chunk/partition); dense
    # one-hot S chunks are built on-device by DVE broadcast compare
    mdl = np.zeros((NC, P, sct), np.float16)
    mwn = np.zeros((NC, P, sct), np.float16)
    for t in range(NT):
        c, tl = t // TPC, t % TPC
        g, ti = tl // G4, tl % G4
        ci0 = 0
        for w in range(4):
            idx = tw_idx[t][w]
            n = len(idx)
            b = int(cb[tl, w]) * P
            arr = np.full(b, idx[-1], np.int16)
            arr[:n] = idx
            wrapped = arr.reshape(-1, 16).T  # [16, b/16]
            coff = int(go4[g, w]) + int(cb[g * G4:tl, w].sum()) * 8
            gidx[c, g, :, coff:coff + b // 16] = np.tile(wrapped, (8, 1))
            sl = np.arange(n)
            mdl[c, sl % P, soff[tl] + ci0 + sl // P] = tw_dl[t][w]
            mwn[c, sl % P, soff[tl] + ci0 + sl // P] = tw_wn[t][w]
            ci0 += int(cb[tl, w])

    # transposed, padded, permuted node features
    xT = np.zeros((FPAD, NPAD), np.float16)
    xT[:IN_F, newpos[:N]] = x.T

    return {
        "newpos": newpos,
        "cb": cb,
        "cb4": cb4,
        "go4": go4,
        "kc": kc,
        "soff": soff,
        "sct": sct,
        "gmax": gmax,
        "xT": xT,
        "mdl": mdl,
        "mwn": mwn,
        "gidx": gidx,
    }


def pack_weights(W1, b1, W2, b2, W3, b3, Wl, bl):
    def chunk_rhs(W, kchunks, dtype):
        # [F, 512] -> [128, kchunks, 512]
        F = W.shape[0]
        Wp = np.zeros((kchunks * P, 512), np.float32)
        Wp[:F] = W
        return np.ascontiguousarray(
            Wp.reshape(kchunks, P, 512).transpose(1, 0, 2)
        ).astype(dtype)

    w1cat = np.concatenate([W1[:IN_F], W1[IN_F:]], axis=1)  # [500, 512]
    w2cat = np.concatenate([W2[:HID], W2[HID:]], axis=1)  # [256, 512]
    w3cat = np.concatenate([W3[:HID], W3[HID:]], axis=1)
    wl = np.ascontiguousarray(
        Wl.reshape(6, P, NCLS).transpose(1, 0, 2)
    ).astype(np.float16)  # [128, 6, 7]
    return {
        "w1": chunk_rhs(w1cat, 4, np.float16),
        "w2": chunk_rhs(w2cat, 2, np.float16),
        "w3": chunk_rhs(w3cat, 2, np.float16),
        "wl": wl,
        "b1": np.tile(b1[None, :], (P, 1)).astype(np.float32),
        "b2": np.tile(b2[None, :], (P, 1)).astype(np.float32),
        "b3": np.tile(b3[None, :], (P, 1)).astype(np.float32),
        "bl": np.tile(bl[None, :], (P, 1)).astype(np.float32),
    }


# --------------------------------------------------------------------------
# device kernel
# --------------------------------------------------------------------------

def build(struct):
    cb = struct["cb"]
    cb4 = struct["cb4"]
    go4 = struct["go4"]
    kc = struct["kc"]
    soff = struct["soff"]
    sct = struct["sct"]
    gmax = struct["gmax"]
    ng4 = TPC // G4
    cb4max = [int(cb4[:, w].max()) for w in range(4)]

    nc = bacc.Bacc(
        "TRN2", target_bir_lowering=False, debug=False, num_devices=NC,
        num_swdge_queues=4,
    )

    xTc = nc.dram_tensor("xTc", [FPAD, NPC], f16, kind="ExternalInput")
    mdl_d = nc.dram_tensor("mdl", [P, sct], f16, kind="ExternalInput")
    mwn_d = nc.dram_tensor("mwn", [P, sct], f16, kind="ExternalInput")
    gidx_d = nc.dram_tensor("gidx", [ng4, P, gmax], i16, kind="ExternalInput")
    w1_d = nc.dram_tensor("w1", [P, 4, 512], f16, kind="ExternalInput")
    w2_d = nc.dram_tensor("w2", [P, 2, 512], f16, kind="ExternalInput")
    w3_d = nc.dram_tensor("w3", [P, 2, 512], f16, kind="ExternalInput")
    wl_d = nc.dram_tensor("wl", [P, 6, NCLS], f16, kind="ExternalInput")
    b1_d = nc.dram_tensor("b1", [P, HID], f32, kind="ExternalInput")
    b2_d = nc.dram_tensor("b2", [P, HID], f32, kind="ExternalInput")
    b3_d = nc.dram_tensor("b3", [P, HID], f32, kind="ExternalInput")
    bl_d = nc.dram_tensor("bl", [P, NCLS], f32, kind="ExternalInput")
    out_d = nc.dram_tensor("out", [NPC, NCLS], f32, kind="ExternalOutput")

    with tile.TileContext(nc) as tc:
        with (
            tc.tile_pool(name="dram", bufs=1, space="DRAM") as dram,
            tc.tile_pool(name="const", bufs=1) as cpool,
            tc.tile_pool(name="lx", bufs=4) as lxpool,
            tc.tile_pool(name="stage", bufs=4) as stpool,
            tc.tile_pool(name="msga", bufs=LA + 1) as msgapool,
            tc.tile_pool(name="msgb", bufs=4) as msgbpool,
            tc.tile_pool(name="sbuild", bufs=2) as sbpool,
            tc.tile_pool(name="psa", bufs=2, space="PSUM") as psa,
            tc.tile_pool(name="psagg", bufs=2, space="PSUM") as psagg,
            tc.tile_pool(name="pstr", bufs=2, space="PSUM") as pstr,
                        tc.tile_pool(name="pscls", bufs=2, space="PSUM") as pscls,
        ):
            # ---- DRAM intermediates
            p_loc = [
                [dram.tile([QROWS, HID], f16, name=f"p{i}loc{h}")
                 for h in range(4)]
                for i in range(3)
            ]
            p_full = [
                [dram.tile([TAB4, HID], f16, addr_space="Shared",
                           name=f"p{i}full{h}") for h in range(4)]
                for i in range(3)
            ]
            r_dram = [dram.tile([NPC, HID], f16, name=f"r{i}d") for i in range(3)]

            # ---- constants
            w1_sb = cpool.tile([P, 4, 512], f16)
            nc.sync.dma_start(w1_sb[:], w1_d[:])
            w2_sb = cpool.tile([P, 2, 512], f16)
            nc.sync.dma_start(w2_sb[:], w2_d[:])
            w3_sb = cpool.tile([P, 2, 512], f16)
            nc.sync.dma_start(w3_sb[:], w3_d[:])
            wl_sb = cpool.tile([P, 6, NCLS], f16)
            nc.sync.dma_start(wl_sb[:], wl_d[:])
            b_sb = []
            for name, t in (("b1", b1_d), ("b2", b2_d), ("b3", b3_d)):
                bt = cpool.tile([P, HID], f32, name=name + "sb")
                nc.sync.dma_start(bt[:], t[:])
                b_sb.append(bt)
            bl_sb = cpool.tile([P, NCLS], f32)
            nc.sync.dma_start(bl_sb[:], bl_d[:])

            gidx_sb = cpool.tile([P, ng4, gmax], i16)
            nc.sync.dma_start(
                gidx_sb[:],
                gidx_d[:].rearrange("g p c -> p g c"),
            )

            ident = cpool.tile([P, P], f16)
            make_identity(nc, ident[:])

            mdl_sb = cpool.tile([P, sct], f16)
            nc.sync.dma_start(mdl_sb[:], mdl_d[:])
            mwn_sb = cpool.tile([P, sct], f16)
            nc.sync.dma_start(mwn_sb[:], mwn_d[:])
            iota_i = cpool.tile([P, P], i32)
            nc.gpsimd.iota(iota_i[:], pattern=[[1, P]], base=0,
                           channel_multiplier=0)
            iota_f = cpool.tile([P, P], f16)
            nc.vector.tensor_copy(iota_f[:], iota_i[:])

            lgall = cpool.tile([P, TPC, NCLS], f32)
            sm_all = cpool.tile([P, TPC], f32)

            # ---- layer-1 phase A (by pairs of tiles): r = x@Wtop+b, p = x@Wbot
            def pair_rows(buf, pr):
                return buf[pr * 2 * P:(pr + 1) * 2 * P, :].rearrange(
                    "(t p) c -> p t c", p=P
                )

            def phase_a_p(pr):
                """p = x @ W1_bot only — unblocks the AllGathers early."""
                half, lpr = pr // (QT // 2), pr % (QT // 2)
                lx = lxpool.tile([P, 4, 2 * P], f16, name="lx", tag="lx")
                nc.sync.dma_start(
                    lx[:],
                    xTc[:, pr * 2 * P:(pr + 1) * 2 * P].rearrange(
                        "(k p) c -> p k c", p=P
                    ),
                )
                pst = stpool.tile([P, 2, HID], f16, name="pst", tag="pst")
                for gt in range(2):
                    ps = psa.tile([P, HID], f32, name="psP", tag="psA")
                    for k in range(4):
                        nc.tensor.matmul(
                            out=ps[:], lhsT=lx[:, k, gt * P:(gt + 1) * P],
                            rhs=w1_sb[:, k, HID:],
                            start=(k == 0), stop=(k == 3),
                        )
                    nc.vector.tensor_copy(pst[:, gt, :], ps[:])
                nc.sync.dma_start(pair_rows(p_loc[0][half], lpr), pst[:])

            def phase_a_r(pr):
                """r = x @ W1_top + b — overlaps the AllGather flight."""
                lx = lxpool.tile([P, 4, 2 * P], f16, name="lx", tag="lx")
                nc.sync.dma_start(
                    lx[:],
                    xTc[:, pr * 2 * P:(pr + 1) * 2 * P].rearrange(
                        "(k p) c -> p k c", p=P
                    ),
                )
                rst = stpool.tile([P, 2, HID], f16, name="rst", tag="rst")
                for gt in range(2):
                    ps = psa.tile([P, HID], f32, name="psP", tag="psA")
                    for k in range(4):
                        nc.tensor.matmul(
                            out=ps[:], lhsT=lx[:, k, gt * P:(gt + 1) * P],
                            rhs=w1_sb[:, k, :HID],
                            start=(k == 0), stop=(k == 3),
                        )
                    nc.vector.tensor_tensor(
                        out=rst[:, gt, :], in0=ps[:], in1=b_sb[0][:],
                        op=mybir.AluOpType.add,
                    )
                nc.sync.dma_start(pair_rows(r_dram[0], pr), rst[:])

            def fire_ag(layer, half):
                nc.gpsimd.collective_compute(
                    "AllGather",
                    mybir.AluOpType.bypass,
                    replica_groups=[list(range(NC))],
                    ins=[p_loc[layer][half].opt()],
                    outs=[p_full[layer][half].opt()],
                )

            # ---- phase B by groups of G4 tiles
            def issue_gathers(layer, g, ws):
                ms = {}
                for w in ws:
                    pool = msgapool if w < 2 else msgbpool
                    m = pool.tile(
                        [P, cb4max[w], HID], f16, name=f"m{w}", tag=f"m{w}"
                    )
                    cols = int(cb4[g, w])
                    ni = cols * P
                    nc.gpsimd.dma_gather(
                        out_ap=m[:, :cols, :],
                        in_ap=p_full[layer][w][:, :],
                        idxs_ap=gidx_sb[:, g, int(go4[g, w]):int(go4[g, w + 1])],
                        num_idxs=ni,
                        num_idxs_reg=ni,
                        elem_size=HID,
                        single_packet=(ni <= 1024),
                        queue_num=w,
                    )
                    ms[w] = m
                return ms

            def phase_b_group(layer, g, msgs):
                last = layer == 2
                t0 = g * G4
                kcg = int(soff[t0 + G4] - soff[t0])
                o0 = int(soff[t0])
                s_v = sbpool.tile([P, kcg, P], f16, name="sv", tag="sv")
                nc.vector.tensor_tensor(
                    out=s_v[:],
                    in0=mdl_sb[:, o0:o0 + kcg].unsqueeze(2).broadcast_to(
                        [P, kcg, P]
                    ),
                    in1=iota_f[:].unsqueeze(1).broadcast_to([P, kcg, P]),
                    op=mybir.AluOpType.is_equal,
                )
                nc.vector.tensor_tensor(
                    out=s_v[:],
                    in0=s_v[:],
                    in1=mwn_sb[:, o0:o0 + kcg].unsqueeze(2).broadcast_to(
                        [P, kcg, P]
                    ),
                    op=mybir.AluOpType.mult,
                )
                rst = stpool.tile([P, G4, HID], f16, name="rl", tag="rl")
                nc.sync.dma_start(
                    rst[:],
                    r_dram[layer][t0 * P:(t0 + G4) * P, :].rearrange(
                        "(t p) c -> p t c", p=P
                    ),
                )
                aggs = []
                for ti in range(G4):
                    tl = t0 + ti
                    agg = psagg.tile([P, HID], f32, name="agg", tag="agg")
                    nchunks = int(kc[tl])
                    cv = int(soff[tl]) - o0
                    done = 0
                    for w in range(4):
                        cbo = int(cb[t0:tl, w].sum())
                        for j in range(int(cb[tl, w])):
                            nc.tensor.matmul(
                                out=agg[:],
                                lhsT=s_v[:, cv, :],
                                rhs=msgs[w][:, cbo + j, :],
                                start=(done == 0),
                                stop=(done == nchunks - 1),
                            )
                            cv += 1
                            done += 1
                    aggs.append(agg)
                for ti in range(G4):
                    tl = t0 + ti
                    agg = aggs[ti]
                    xsum = stpool.tile([P, HID], f16, name="xsum", tag="xsum")
                    nc.vector.tensor_tensor(
                        out=xsum[:], in0=agg[:], in1=rst[:, ti, :],
                        op=mybir.AluOpType.add,
                    )
                    xn = stpool.tile([P, HID], f16, name="xn", tag="xn")
                    nc.scalar.activation(
                        xn[:], xsum[:], mybir.ActivationFunctionType.Relu
                    )
                    xts = []
                    for h in range(2):
                        tp = pstr.tile([P, P], f16, name="tp", tag="tp")
                        nc.tensor.transpose(
                            out=tp[:], in_=xn[:, h * P:(h + 1) * P],
                            identity=ident[:],
                        )
                        xt = stpool.tile([P, P], f16, name=f"xt{h}",
                                         tag=f"xt{h}")
                        nc.vector.tensor_copy(xt[:], tp[:])
                        xts.append(xt)
                    # fused next-layer phase A: r/p for layer+1 from x in SBUF
                    if not last:
                        half, lt = tl // QT, tl % QT
                        wsb = w2_sb if layer == 0 else w3_sb
                        ps = psa.tile([P, 512], f32, name="psA", tag="psA")
                        for k in range(2):
                            nc.tensor.matmul(
                                out=ps[:], lhsT=xts[k][:], rhs=wsb[:, k, :],
                                start=(k == 0), stop=(k == 1),
                            )
                        rs2 = stpool.tile([P, HID], f16, name="rs2", tag="rs2")
                        nc.vector.tensor_tensor(
                            out=rs2[:], in0=ps[:, :HID], in1=b_sb[layer + 1][:],
                            op=mybir.AluOpType.add,
                        )
                        ps2 = stpool.tile([P, HID], f16, name="ps2", tag="ps2")
                        nc.vector.tensor_copy(ps2[:], ps[:, HID:])
                        nc.sync.dma_start(
                            r_dram[layer + 1][tl * P:(tl + 1) * P, :].rearrange(
                                "(o p) c -> p (o c)", p=P
                            ),
                            rs2[:],
                        )
                        nc.sync.dma_start(
                            p_loc[layer + 1][half][
                                lt * P:(lt + 1) * P, :
                            ].rearrange("(o p) c -> p (o c)", p=P),
                            ps2[:],
                        )
                    # incremental classifier: lgall += x_l^T @ Wl[2l:2l+2]
                    cls = pscls.tile([P, NCLS], f32, name="cls", tag="cls")
                    for kk in range(2):
                        nc.tensor.matmul(
                            out=cls[:], lhsT=xts[kk][:],
                            rhs=wl_sb[:, 2 * layer + kk, :],
                            start=(kk == 0), stop=(kk == 1),
                        )
                    if layer == 0:
                        nc.vector.tensor_copy(lgall[:, tl, :], cls[:])
                    else:
                        nc.vector.tensor_tensor(
                            out=lgall[:, tl, :], in0=lgall[:, tl, :],
                            in1=cls[:], op=mybir.AluOpType.add,
                        )
                    if last:
                        nc.vector.tensor_tensor(
                            out=lgall[:, tl, :], in0=lgall[:, tl, :],
                            in1=bl_sb[:], op=mybir.AluOpType.add,
                        )

            # batched shifted-exp over a block of tiles (layer 3)
            def softmax_block(t0, nt):
                mx = stpool.tile([P, nt], f32, name="mx", tag="mx")
                nc.vector.tensor_reduce(
                    out=mx[:], in_=lgall[:, t0:t0 + nt, :],
                    axis=mybir.AxisListType.X, op=mybir.AluOpType.max,
                )
                nc.vector.tensor_tensor(
                    out=lgall[:, t0:t0 + nt, :], in0=lgall[:, t0:t0 + nt, :],
                    in1=mx[:].unsqueeze(2).broadcast_to([P, nt, NCLS]),
                    op=mybir.AluOpType.subtract,
                )
                ex = stpool.tile([P, nt, NCLS], f32, name="ex", tag="ex")
                nc.scalar.activation(
                    ex[:], lgall[:, t0:t0 + nt, :],
                    mybir.ActivationFunctionType.Exp,
                )
                nc.vector.tensor_reduce(
                    out=sm_all[:, t0:t0 + nt], in_=ex[:],
                    axis=mybir.AxisListType.X, op=mybir.AluOpType.add,
                )

            # ---- drive: layer-1 phase A (p first, AGs early, r under AG),
            # then 3 fused phase-B passes
            for pr in range(HT):
                phase_a_p(pr)
                if pr % (QT // 2) == QT // 2 - 1:
                    fire_ag(0, pr // (QT // 2))
            for pr in range(HT):
                phase_a_r(pr)
            for layer in range(3):
                # lookahead gathers: windows 0/1 run LA groups ahead (their
                # quarter AGs finish first); windows 2/3 run one group ahead
                # so their desc-gen + DMA latency stays off the critical path
                amsgs = [issue_gathers(layer, g, (0, 1)) for g in range(LA)]
                bmsgs = [issue_gathers(layer, 0, (2, 3))]
                for g in range(ng4):
                    if g + 1 < ng4:
                        bmsgs.append(issue_gathers(layer, g + 1, (2, 3)))
                    if g + LA < ng4:
                        amsgs.append(issue_gathers(layer, g + LA, (0, 1)))
                    msgs = dict(bmsgs[g])
                    msgs.update(amsgs[g])
                    phase_b_group(layer, g, msgs)
                    if layer < 2:
                        if g in (5, 10, 16, 21):
                            fire_ag(layer + 1, {5: 0, 10: 1, 16: 2, 21: 3}[g])
                    else:
                        softmax_block(g * G4, G4)

            lsm = cpool.tile([P, TPC], f32)
            nc.scalar.activation(
                lsm[:], sm_all[:], mybir.ActivationFunctionType.Ln
            )
            nc.vector.tensor_tensor(
                out=lgall[:], in0=lgall[:],
                in1=lsm[:].unsqueeze(2).broadcast_to([P, TPC, NCLS]),
                op=mybir.AluOpType.subtract,
            )
            nc.sync.dma_start(
                out_d[:].rearrange("(t p) j -> p t j", p=P), lgall[:]
            )

    nc.compile()
    return nc


# --------------------------------------------------------------------------
# entry point
# --------------------------------------------------------------------------

def kernel(x, edge_index, edge_weight, W1, b1, W2, b2, W3, b3, Wl, bl):
    x = np.asarray(x, dtype=np.float32)
    edge_index = np.asarray(edge_index)
    edge_weight = np.asarray(edge_weight, dtype=np.float32)

    pp = prep(x, edge_index, edge_weight)
    wts = pack_weights(
        np.asarray(W1, np.float32), np.asarray(b1, np.float32),
        np.asarray(W2, np.float32), np.asarray(b2, np.float32),
        np.asarray(W3, np.float32), np.asarray(b3, np.float32),
        np.asarray(Wl, np.float32), np.asarray(bl, np.float32),
    )

    key = (pp["sct"], pp["gmax"], tuple(pp["cb"].reshape(-1).tolist()))
    if key not in _compile_cache:
        _compile_cache[key] = build(pp)
    nc = _compile_cache[key]

    in_maps = []
    for c in range(NC):
        in_maps.append({
            "xTc": np.ascontiguousarray(pp["xT"][:, c * NPC:(c + 1) * NPC]),
            "mdl": np.ascontiguousarray(pp["mdl"][c]),
            "mwn": np.ascontiguousarray(pp["mwn"][c]),
            "gidx": np.ascontiguousarray(pp["gidx"][c]),
            **wts,
        })

    res = run_bass_kernel_spmd(nc, in_maps, list(range(NC)))
    out_full = np.concatenate([res.results[c]["out"] for c in range(NC)], axis=0)
    return out_full[pp["newpos"][:N]].astype(np.float32)


if __name__ == "__main__":
    import time

    rng = np.random.default_rng(0)
    E = 899756
    ei = rng.integers(0, N, (2, E)).astype(np.int32)
    ew = rng.random(E, dtype=np.float32)
    x = rng.standard_normal((N, IN_F), dtype=np.float32)
    t0 = time.time()
    pp = prep(x, ei, ew)
    print("prep", time.time() - t0, "sct =", pp["sct"], "gmax =", pp["gmax"])
